# revision 1
# baseline (speedup 1.0000x reference)
"""GCN link-decoder kernel for 8 TRN2 NeuronCores.

Math: both GCNConv layers are linear (no activation), so with
P = D^-1/2 (A+I) D^-1/2 the network output is
    value_e = sigmoid( h2[src_e] . h2[dst_e] ),  h2 = P^2 z W1 W2  (b1=b2=0)
which reduces to 16-dim aggregations:
    t0 = dinv * z;  agg1 = sum_{e into d} t0[src_e];  t1 = dinv^2*(agg1 + t0)
    agg2 = sum t1[src_e];  u = dinv*(agg2 + t1);  v = u @ G,  G = (W1W2)(W1W2)^T
    value_e = v[src_e] . u[dst_e]
Nodes are range-sharded across the 8 cores (dst owner aggregates); the
16-wide node tables are replicated between phases with AllGather; the
per-edge gathers run on the SWDGE dma_gather unit (256B rows, int16
bucketed indices); scatter-add is a one-hot (is_equal vs iota) matmul
accumulated in PSUM per (bucket, dst-tile) cell.
"""
import sys
import os
import bisect
sys.path.insert(0, '/opt/trn_rl_repo')
import numpy as np

NC = 8          # cores
P = 128         # partitions / chunk size
FW = 64         # table row width in f32 (256B dma_gather granule)
BUCK = 32768    # int16 index bucket size (table rows per bucket)
BLK = 8192      # gather idxs per dma_gather instruction


def _wrap_idx16(arr: np.ndarray) -> np.ndarray:
    """Linear int16 slot-index array (len % 128 == 0) -> [128, len/16] SWDGE
    wrapped layout (slot k at partition k%16, col k//16; 16-row pattern
    replicated to 128 partitions)."""
    n = arr.shape[0]
    t16 = arr.reshape(n // 16, 16).T
    return np.ascontiguousarray(np.tile(t16, (8, 1)))


def _host_reference(z, edge_index, W1, b1, W2, b2):
    """Numpy fallback (used only when b1/b2 are nonzero)."""
    N = z.shape[0]
    src, dst = edge_index[0], edge_index[1]
    deg = (np.bincount(dst, minlength=N) + 1.0).astype(np.float64)
    dinv = (1.0 / np.sqrt(deg)).astype(np.float32)

    def conv(x, W, b):
        h = x @ W
        out = np.zeros_like(h)
        np.add.at(out, dst, h[src] * (dinv[src] * dinv[dst])[:, None])
        out += h * (dinv * dinv)[:, None]
        return out + b

    h = conv(z, W1, b1)
    h = conv(h, W2, b2)
    val = np.einsum('ef,ef->e', h[src], h[dst]).astype(np.float64)
    return (1.0 / (1.0 + np.exp(-val))).astype(np.float32)


def _plan(z, edge_index):
    """Host-side layout planning: shard nodes/edges, build slot arrays."""
    N = z.shape[0]
    E = edge_index.shape[1]
    assert N % NC == 0 and E % NC == 0
    npc = N // NC                      # real nodes per core
    npad = ((npc + P - 1) // P) * P    # padded nodes per core
    tiles = npad // P
    nrows = NC * npad                  # table rows
    nbuck = (nrows + BUCK - 1) // BUCK

    src = edge_index[0].astype(np.int64)
    dst = edge_index[1].astype(np.int64)
    deg = np.bincount(dst, minlength=N).astype(np.float64) + 1.0
    dinv = (1.0 / np.sqrt(deg)).astype(np.float32)

    owner_s, local_s = src // npc, src % npc
    owner_d, local_d = dst // npc, dst % npc
    pid_s = (owner_s * npad + local_s).astype(np.int64)
    pid_d = (owner_d * npad + local_d).astype(np.int64)
    b_s = (pid_s // BUCK).astype(np.int64)
    b_d = (pid_d // BUCK).astype(np.int64)

    plan = {
        'N': N, 'E': E, 'npc': npc, 'npad': npad, 'tiles': tiles,
        'nrows': nrows, 'nbuck': nbuck, 'dinv': dinv,
    }

    # ---------------- aggregation slots (per dst-owner core) --------------
    # cell = (bucket(src), dst_tile); bucket-major order.
    t_d = local_d // P                     # dst tile within owner
    cell = b_s * tiles + t_d               # cell id within owner core
    ncell = nbuck * tiles
    counts = np.zeros((NC, ncell), np.int64)
    for c in range(NC):
        m = owner_d == c
        counts[c] = np.bincount(cell[m], minlength=ncell)
    K = np.maximum(np.ceil(counts.max(axis=0) / P).astype(np.int64), 0)  # chunks per cell
    cell_slots = K * P
    cell_ofs = np.concatenate([[0], np.cumsum(cell_slots)])
    tot_agg = int(cell_ofs[-1])
    plan['K'] = K
    plan['cell_ofs'] = cell_ofs
    plan['tot_agg'] = tot_agg

    agg_idx = np.zeros((NC, 128, tot_agg // 16), np.int16)
    agg_dstloc = np.full((NC, 128, tot_agg // 128), -1.0, np.float32)
    for c in range(NC):
        m = owner_d == c
        cl = cell[m]
        order = np.argsort(cl, kind='stable')
        cl_s = cl[order]
        # rank within cell
        grp_start = np.searchsorted(cl_s, np.arange(ncell))
        rank = np.arange(cl_s.shape[0]) - grp_start[cl_s]
        slot = cell_ofs[cl_s] + rank
        idx_lin = np.zeros(tot_agg, np.int16)
        dl_lin = np.full(tot_agg, -1.0, np.float32)
        ps = pid_s[m][order]
        idx_lin[slot] = (ps - (ps // BUCK) * BUCK).astype(np.int16)
        dl_lin[slot] = (local_d[m][order] % P).astype(np.float32)
        agg_idx[c] = _wrap_idx16(idx_lin)
        agg_dstloc[c] = np.ascontiguousarray(dl_lin.reshape(-1, 128).T)
    plan['agg_idx'] = agg_idx
    plan['agg_dstloc'] = agg_dstloc

    # gather blocks: contiguous slot ranges within one src bucket
    blocks = []  # (bucket, slot_start, n_idxs)
    for b in range(nbuck):
        s0 = int(cell_ofs[b * tiles])
        s1 = int(cell_ofs[(b + 1) * tiles])
        s = s0
        while s < s1:
            n = min(BLK, s1 - s)
            blocks.append((b, s, n))
            s += n
    plan['agg_blocks'] = blocks

    # ---------------- scoring slots (per original-order core split) -------
    epc = E // NC
    seg = (b_s * nbuck + b_d).astype(np.int64)   # segment id
    nseg = nbuck * nbuck
    scnt = np.zeros((NC, nseg), np.int64)
    for c in range(NC):
        scnt[c] = np.bincount(seg[c * epc:(c + 1) * epc], minlength=nseg)
    SEG = (np.ceil(scnt.max(axis=0) / P) * P).astype(np.int64)
    seg_ofs = np.concatenate([[0], np.cumsum(SEG)])
    tot_sc = int(seg_ofs[-1])
    totc = tot_sc // P
    totc_pad = ((totc + P - 1) // P) * P
    plan['SEG'] = SEG
    plan['seg_ofs'] = seg_ofs
    plan['tot_sc'] = tot_sc
    plan['totc_pad'] = totc_pad

    sc_src = np.zeros((NC, 128, tot_sc // 16), np.int16)
    sc_dst = np.zeros((NC, 128, tot_sc // 16), np.int16)
    sc_perm = np.zeros((NC, epc), np.int64)     # slot of edge i (within core)
    for c in range(NC):
        sl = seg[c * epc:(c + 1) * epc]
        order = np.argsort(sl, kind='stable')
        sl_s = sl[order]
        grp_start = np.searchsorted(sl_s, np.arange(nseg))
        rank = np.arange(epc) - grp_start[sl_s]
        slot = seg_ofs[sl_s] + rank
        sc_perm[c][order] = slot
        si = np.zeros(tot_sc, np.int16)
        di = np.zeros(tot_sc, np.int16)
        ps = pid_s[c * epc:(c + 1) * epc][order]
        pd = pid_d[c * epc:(c + 1) * epc][order]
        si[slot] = (ps - (ps // BUCK) * BUCK).astype(np.int16)
        di[slot] = (pd - (pd // BUCK) * BUCK).astype(np.int16)
        sc_src[c] = _wrap_idx16(si)
        sc_dst[c] = _wrap_idx16(di)
    plan['sc_src'] = sc_src
    plan['sc_dst'] = sc_dst
    plan['sc_perm'] = sc_perm

    sblocks = []  # (b_src, b_dst, slot_start, n_idxs)
    for b1 in range(nbuck):
        for b2 in range(nbuck):
            s0 = int(seg_ofs[b1 * nbuck + b2])
            s1 = int(seg_ofs[b1 * nbuck + b2 + 1])
            s = s0
            while s < s1:
                n = min(BLK, s1 - s)
                sblocks.append((b1, b2, s, n))
                s += n
    plan['sc_blocks'] = sblocks

    # ---------------- per-core node data ----------------------------------
    z_cols = np.zeros((NC, 128, (npad // P) * 16), np.float32)
    dinv_cols = np.zeros((NC, 128, npad // P), np.float32)
    for c in range(NC):
        zc = np.zeros((npad, 16), np.float32)
        zc[:npc] = z[c * npc:(c + 1) * npc]
        dc = np.zeros(npad, np.float32)
        dc[:npc] = dinv[c * npc:(c + 1) * npc]
        # [npad,16] -> [128, tiles, 16] with node t*128+p at [p, t, :]
        z_cols[c] = zc.reshape(tiles, P, 16).transpose(1, 0, 2).reshape(P, tiles * 16)
        dinv_cols[c] = dc.reshape(tiles, P).T
    plan['z_cols'] = z_cols
    plan['dinv_cols'] = dinv_cols
    plan['dinv2_cols'] = dinv_cols * dinv_cols
    return plan


def _build(plan, W1np, W2np):
    """Build + compile the SPMD bass program (same program for all cores)."""
    from concourse import bass, bacc, tile, mybir
    from concourse.masks import make_identity

    npad, tiles, nrows, nbuck = plan['npad'], plan['tiles'], plan['nrows'], plan['nbuck']
    tot_agg, tot_sc = plan['tot_agg'], plan['tot_sc']
    totc_pad = plan['totc_pad']
    K, cell_ofs = plan['K'], plan['cell_ofs']
    f32 = mybir.dt.float32

    nc = bacc.Bacc("TRN2", target_bir_lowering=False, debug=False, num_devices=NC)

    # ---- I/O ----
    in_z = nc.dram_tensor("z_cols", [128, tiles * 16], f32, kind="ExternalInput")
    in_dinv = nc.dram_tensor("dinv_cols", [128, tiles], f32, kind="ExternalInput")
    in_dinv2 = nc.dram_tensor("dinv2_cols", [128, tiles], f32, kind="ExternalInput")
    in_w1t = nc.dram_tensor("w1t", [256, 16], f32, kind="ExternalInput")
    in_w2 = nc.dram_tensor("w2", [256, 256], f32, kind="ExternalInput")
    in_aidx = nc.dram_tensor("agg_idx", [128, tot_agg // 16], mybir.dt.int16, kind="ExternalInput")
    in_adl = nc.dram_tensor("agg_dstloc", [128, tot_agg // 128], f32, kind="ExternalInput")
    in_ssrc = nc.dram_tensor("sc_src", [128, tot_sc // 16], mybir.dt.int16, kind="ExternalInput")
    in_sdst = nc.dram_tensor("sc_dst", [128, tot_sc // 16], mybir.dt.int16, kind="ExternalInput")
    in_iota = nc.dram_tensor("iota_row", [128, 128], f32, kind="ExternalInput")
    out_val = nc.dram_tensor("out_val", [totc_pad, 128], f32, kind="ExternalOutput")

    with tile.TileContext(nc) as tc:
        with tc.tile_pool(name="res", bufs=1) as res, \
             tc.tile_pool(name="gat", bufs=2) as gat, \
             tc.tile_pool(name="idx", bufs=2) as idxp, \
             tc.tile_pool(name="oh", bufs=2) as ohp, \
             tc.tile_pool(name="sm", bufs=3) as sm, \
             tc.tile_pool(name="ps", bufs=4, space="PSUM") as ps, \
             tc.tile_pool(name="pst", bufs=2, space="PSUM") as pst, \
             tc.tile_pool(name="dram", bufs=1, space="DRAM") as dram:

            # ================= phase 0: constants, zt, G ==================
            ident = res.tile([128, 128], f32)
            make_identity(nc, ident[:])
            iota = res.tile([128, 128], f32)     # iota along free dim, same per partition
            nc.sync.dma_start(iota[:], in_iota[:])

            dinv_t = res.tile([128, tiles], f32)
            nc.sync.dma_start(dinv_t[:], in_dinv[:])
            dinv2_t = res.tile([128, tiles], f32)
            nc.sync.dma_start(dinv2_t[:], in_dinv2[:])

            zt = res.tile([128, tiles * 16], f32)
            nc.sync.dma_start(zt[:], in_z[:])
            nc.vector.tensor_tensor(
                out=zt[:].rearrange("p (t f) -> p t f", f=16),
                in0=zt[:].rearrange("p (t f) -> p t f", f=16),
                in1=dinv_t[:][:, :, None].to_broadcast([128, tiles, 16]),
                op=mybir.AluOpType.mult)

            # G = (W1 @ W2) @ (W1 @ W2)^T  [16,16]
            w1t_s = res.tile([128, 2 * 16], f32)     # two 128-row blocks of W1T side by side
            nc.sync.dma_start(w1t_s[:, 0:16], in_w1t[0:128, :])
            nc.sync.dma_start(w1t_s[:, 16:32], in_w1t[128:256, :])
            w2_s = res.tile([128, 2 * 256], f32)
            nc.sync.dma_start(w2_s[:, 0:256], in_w2[0:128, :])
            nc.sync.dma_start(w2_s[:, 256:512], in_w2[128:256, :])
            w12_ps = pst.tile([16, 256], f32, tag="tp", space="PSUM")
            nc.tensor.matmul(w12_ps[:], lhsT=w1t_s[:, 0:16], rhs=w2_s[:, 0:256], start=True, stop=False)
            nc.tensor.matmul(w12_ps[:], lhsT=w1t_s[:, 16:32], rhs=w2_s[:, 256:512], start=False, stop=True)
            w12_s = res.tile([16, 256], f32)
            nc.vector.tensor_copy(w12_s[:], w12_ps[:])
            # transpose W12 -> [256,16] in two blocks
            w12T_s = res.tile([128, 2 * 16], f32)
            for blkk in range(2):
                tp = pst.tile([128, 16], f32, tag="tp", space="PSUM")
                nc.tensor.transpose(tp[:], in_=w12_s[:, blkk * 128:(blkk + 1) * 128], identity=ident[:16, :16])
                nc.vector.tensor_copy(w12T_s[:, blkk * 16:(blkk + 1) * 16], tp[:])
            g_ps = pst.tile([16, 16], f32, tag="tp", space="PSUM")
            nc.tensor.matmul(g_ps[:], lhsT=w12T_s[:, 0:16], rhs=w12T_s[:, 0:16], start=True, stop=False)
            nc.tensor.matmul(g_ps[:], lhsT=w12T_s[:, 16:32], rhs=w12T_s[:, 16:32], start=False, stop=True)
            g_s = res.tile([16, 16], f32)
            nc.vector.tensor_copy(g_s[:], g_ps[:])

            # ---- zt -> table + AllGather ----
            def table_write(sbuf_cols, bounce):
                # sbuf [128, tiles*16] -> dram [npad, 64] rows (cols 0:16)
                dst = bounce[:].rearrange("(t p) (a f) -> p t a f", p=128, a=4)[:, :, 0, :]
                nc.sync.dma_start(dst, sbuf_cols[:].rearrange("p (t f) -> p t f", f=16))

            rg = [list(range(NC))]
            ztb = dram.tile([npad, FW], f32)
            zt_full = dram.tile([nrows, FW], f32)
            table_write(zt, ztb)
            nc.gpsimd.collective_compute(
                "AllGather", mybir.AluOpType.bypass,
                ins=[ztb.opt()], outs=[zt_full.opt()], replica_groups=rg)

            # ================= aggregation layer ==========================
            acc = res.tile([128, tiles * 16], f32)
            t1 = res.tile([128, tiles * 16], f32)
            adl_t = res.tile([128, tot_agg // 128], f32)
            nc.sync.dma_start(adl_t[:], in_adl[:])

            def agg_layer(table_full, out_sb, scale_t, selfloop_sb):
                """out_sb = scale ⊙ (scatter-sum(table[src]) + selfloop)"""
                nc.vector.memset(acc[:], 0.0)
                # emit gathers lazily as chunks consume them
                blk_tiles = {}

                def get_block(bi):
                    if bi in blk_tiles:
                        return blk_tiles[bi]
                    b, s0, n = plan['agg_blocks'][bi]
                    it = idxp.tile([128, BLK // 16], mybir.dt.int16, tag="aggidx")
                    nc.sync.dma_start(it[:, :n // 16], in_aidx[:, s0 // 16:(s0 + n) // 16])
                    gt = gat.tile([128, (BLK // 128) * FW], f32, tag="aggbuf")
                    lo = b * BUCK
                    hi = min(lo + BUCK, nrows)
                    nc.gpsimd.dma_gather(
                        out_ap=gt[:, :(n // 128) * FW].rearrange("p (c f) -> p c f", f=FW),
                        in_ap=table_full[lo:hi, :],
                        idxs_ap=it[:, :n // 16],
                        num_idxs=n, num_idxs_reg=n, elem_size=FW,
                        single_packet=False)
                    blk_tiles[bi] = (gt, s0, n)
                    return blk_tiles[bi]

                # map slot -> block index
                bstarts = [b[1] for b in plan['agg_blocks']]
                for b in range(nbuck):
                    for t in range(tiles):
                        kk = int(K[b * tiles + t])
                        if kk == 0:
                            continue
                        c0 = int(cell_ofs[b * tiles + t])
                        pt = ps.tile([128, 16], f32, tag="mm", space="PSUM")
                        # one-hot for the whole cell: [128, kk*128]
                        oh = ohp.tile([128, kk * 128], f32, tag="oh")
                        nc.vector.tensor_tensor(
                            out=oh[:].rearrange("p (k q) -> p k q", q=128),
                            in0=iota[:][:, None, :].to_broadcast([128, kk, 128]),
                            in1=adl_t[:, c0 // 128:c0 // 128 + kk][:, :, None]
                                .to_broadcast([128, kk, 128]),
                            op=mybir.AluOpType.is_equal)
                        for j in range(kk):
                            slot = c0 + j * 128
                            bi = bisect.bisect_right(bstarts, slot) - 1
                            gt, s0, n = get_block(bi)
                            ch = (slot - s0) // 128
                            nc.tensor.matmul(
                                pt[:],
                                lhsT=oh[:, j * 128:(j + 1) * 128],
                                rhs=gt[:].rearrange("p (c f) -> p c f", f=FW)[:, ch, 0:16],
                                start=(j == 0), stop=(j == kk - 1))
                        nc.vector.tensor_add(
                            out=acc[:, t * 16:(t + 1) * 16],
                            in0=acc[:, t * 16:(t + 1) * 16], in1=pt[:])
                # epilogue: out = scale ⊙ (acc + selfloop)
                nc.vector.tensor_add(out=out_sb[:], in0=acc[:], in1=selfloop_sb[:])
                nc.vector.tensor_tensor(
                    out=out_sb[:].rearrange("p (t f) -> p t f", f=16),
                    in0=out_sb[:].rearrange("p (t f) -> p t f", f=16),
                    in1=scale_t[:][:, :, None].to_broadcast([128, tiles, 16]),
                    op=mybir.AluOpType.mult)

            # L1: t1 = dinv2 ⊙ (agg(zt) + zt)
            agg_layer(zt_full, t1, dinv2_t, zt)
            t1b = dram.tile([npad, FW], f32)
            t1_full = dram.tile([nrows, FW], f32)
            table_write(t1, t1b)
            nc.gpsimd.collective_compute(
                "AllGather", mybir.AluOpType.bypass,
                ins=[t1b.opt()], outs=[t1_full.opt()], replica_groups=rg)

            # L2: u = dinv ⊙ (agg(t1) + t1)
            u_sb = res.tile([128, tiles * 16], f32)
            agg_layer(t1_full, u_sb, dinv_t, t1)

            # v = u @ G per tile
            v_sb = res.tile([128, tiles * 16], f32)
            for t in range(tiles):
                tp = pst.tile([16, 128], f32, tag="tp", space="PSUM")
                nc.tensor.transpose(tp[:], in_=u_sb[:, t * 16:(t + 1) * 16], identity=ident[:])
                uT = sm.tile([16, 128], f32, tag="uTs")
                nc.vector.tensor_copy(uT[:], tp[:])
                vp = ps.tile([128, 16], f32, tag="mm", space="PSUM")
                nc.tensor.matmul(vp[:], lhsT=uT[:], rhs=g_s[:], start=True, stop=True)
                nc.vector.tensor_copy(v_sb[:, t * 16:(t + 1) * 16], vp[:])

            vb = dram.tile([npad, FW], f32)
            ub = dram.tile([npad, FW], f32)
            vtab = dram.tile([nrows, FW], f32)
            utab = dram.tile([nrows, FW], f32)
            table_write(v_sb, vb)
            table_write(u_sb, ub)
            nc.gpsimd.collective_compute(
                "AllGather", mybir.AluOpType.bypass,
                ins=[vb.opt()], outs=[vtab.opt()], replica_groups=rg)
            nc.gpsimd.collective_compute(
                "AllGather", mybir.AluOpType.bypass,
                ins=[ub.opt()], outs=[utab.opt()], replica_groups=rg)

            # ================= scoring ====================================
            val = res.tile([128, totc_pad], f32)
            nc.vector.memset(val[:], 0.0)
            for (b1, b2, s0, n) in plan['sc_blocks']:
                itv = idxp.tile([128, BLK // 16], mybir.dt.int16, tag="scidxv")
                nc.sync.dma_start(itv[:, :n // 16], in_ssrc[:, s0 // 16:(s0 + n) // 16])
                itu = idxp.tile([128, BLK // 16], mybir.dt.int16, tag="scidxu")
                nc.sync.dma_start(itu[:, :n // 16], in_sdst[:, s0 // 16:(s0 + n) // 16])
                gv = gat.tile([128, (BLK // 128) * FW], f32, tag="aggbuf")
                gu = gat.tile([128, (BLK // 128) * FW], f32, tag="scubuf")
                lo1 = b1 * BUCK
                lo2 = b2 * BUCK
                nc.gpsimd.dma_gather(
                    out_ap=gv[:, :(n // 128) * FW].rearrange("p (c f) -> p c f", f=FW),
                    in_ap=vtab[lo1:min(lo1 + BUCK, nrows), :],
                    idxs_ap=itv[:, :n // 16],
                    num_idxs=n, num_idxs_reg=n, elem_size=FW, single_packet=False)
                nc.gpsimd.dma_gather(
                    out_ap=gu[:, :(n // 128) * FW].rearrange("p (c f) -> p c f", f=FW),
                    in_ap=utab[lo2:min(lo2 + BUCK, nrows), :],
                    idxs_ap=itu[:, :n // 16],
                    num_idxs=n, num_idxs_reg=n, elem_size=FW, single_packet=False)
                nch = n // 128
                prod = sm.tile([128, (BLK // 128) * 16], f32, tag="prod")
                nc.vector.tensor_tensor(
                    out=prod[:, :nch * 16].rearrange("p (c f) -> p c f", f=16),
                    in0=gv[:].rearrange("p (c f) -> p c f", f=FW)[:, 0:nch, 0:16],
                    in1=gu[:].rearrange("p (c f) -> p c f", f=FW)[:, 0:nch, 0:16],
                    op=mybir.AluOpType.mult)
                nc.vector.reduce_sum(
                    out=val[:, s0 // 128:s0 // 128 + nch],
                    in_=prod[:, :nch * 16].rearrange("p (c f) -> p c f", f=16),
                    axis=mybir.AxisListType.X)

            # sigmoid + transpose + out
            for g in range(totc_pad // 128):
                sg = sm.tile([128, 128], f32, tag="sig")
                nc.scalar.activation(sg[:], val[:, g * 128:(g + 1) * 128],
                                     mybir.ActivationFunctionType.Sigmoid)
                tp = pst.tile([128, 128], f32, tag="tp", space="PSUM")
                nc.tensor.transpose(tp[:], in_=sg[:], identity=ident[:])
                so = sm.tile([128, 128], f32, tag="sigT")
                nc.vector.tensor_copy(so[:], tp[:])
                nc.sync.dma_start(out_val[g * 128:(g + 1) * 128, :], so[:])

    nc.compile()
    return nc


_CACHE = {}


def kernel(z, edge_index, W1, b1, W2, b2):
    z = np.asarray(z, np.float32)
    edge_index = np.asarray(edge_index)
    W1 = np.asarray(W1, np.float32)
    W2 = np.asarray(W2, np.float32)
    b1 = np.asarray(b1, np.float32)
    b2 = np.asarray(b2, np.float32)
    if np.any(b1 != 0) or np.any(b2 != 0):
        return _host_reference(z, edge_index, W1, b1, W2, b2)

    from concourse import bass_utils

    plan = _plan(z, edge_index)
    key = (z.shape, edge_index.shape, plan['tot_agg'], plan['tot_sc'],
           tuple(plan['K'].tolist()), tuple(plan['SEG'].tolist()))
    if key not in _CACHE:
        _CACHE.clear()
        _CACHE[key] = _build(plan, W1, W2)
    nc = _CACHE[key]

    w1t = np.ascontiguousarray(W1.T)
    in_maps = []
    for c in range(NC):
        in_maps.append({
            "z_cols": plan['z_cols'][c],
            "dinv_cols": plan['dinv_cols'][c],
            "dinv2_cols": plan['dinv2_cols'][c],
            "w1t": w1t, "w2": W2,
            "agg_idx": plan['agg_idx'][c],
            "agg_dstloc": plan['agg_dstloc'][c],
            "sc_src": plan['sc_src'][c],
            "sc_dst": plan['sc_dst'][c],
            "iota_row": np.ascontiguousarray(
                np.tile(np.arange(128, dtype=np.float32), (128, 1))),
        })
    res = bass_utils.run_bass_kernel_spmd(nc, in_maps, core_ids=list(range(NC)))
    kernel._last = (nc, in_maps, plan)

    E = plan['E']
    epc = E // NC
    out = np.empty(E, np.float32)
    for c in range(NC):
        flat = res.results[c]["out_val"].reshape(-1)
        out[c * epc:(c + 1) * epc] = flat[plan['sc_perm'][c]]
    return out



# revision 2
# speedup vs baseline: 1.1374x; 1.1374x over previous
"""GCN link-decoder kernel for 8 TRN2 NeuronCores — v2 (SWDGE-queue parallel).

Math (both GCNConv layers are linear, b1=b2=0): with P = D^-1/2 (A+I) D^-1/2,
    t0 = dinv*z; agg1 = scatter-sum t0[src]; t1 = dinv^2*(agg1 + t0)
    agg2 = scatter-sum t1[src]; u = dinv*(agg2 + t1); w = u @ G, G=(W1W2)(W1W2)^T
    val_e = u[src_e] . w[dst_e];  out = sigmoid(val)

Design notes (from the profiled baseline, 17.0 ms):
- The bottleneck was SWDGE descriptor generation on the GpSimd engine
  (~7.8 ns/gather-index, 13.1 ms of Pool time). v2 cuts per-edge gather
  streams from 4 to 3 (L1, L2, score-u) by computing the dst-side score
  factor w[dst_e] with a one-hot expansion matmul on TensorE, and spreads
  the remaining gathers across all 4 SWDGE queues, whose descriptor
  generation runs concurrently (measured 3-4x).
- Everything per-edge is processed in ONE slot layout shared by all three
  phases: cell = (src bucket, dst tile) at the dst-owner core, bucket-major
  slots, t-major processing (PSUM accumulates the 4 buckets per dst tile).
- Tables are bf16 (logits are in [-0.5, 0.7], so bf16 end-to-end error is
  ~1e-3 abs, far inside the 2e-2 gate); one-hots bf16 (2x DVE is_equal,
  FWL weight loads).
"""
import sys
import os
import bisect
sys.path.insert(0, '/opt/trn_rl_repo')
import numpy as np

NC = 8          # cores
P = 128         # partitions
FWH = 128       # table row width in bf16 elems (256B dma_gather granule)
BUCK = 32768    # int16 index bucket size (table rows per bucket)
BLK = 4096      # gather idxs per dma_gather instruction


def _wrap_idx16(arr: np.ndarray) -> np.ndarray:
    """Linear int16 slot-index array (len % 128 == 0) -> [128, len/16] SWDGE
    wrapped layout (slot k at partition k%16, col k//16; replicated to 128)."""
    n = arr.shape[0]
    t16 = arr.reshape(n // 16, 16).T
    return np.ascontiguousarray(np.tile(t16, (8, 1)))


def _host_reference(z, edge_index, W1, b1, W2, b2):
    """Numpy fallback (used only when b1/b2 are nonzero)."""
    N = z.shape[0]
    src, dst = edge_index[0], edge_index[1]
    deg = (np.bincount(dst, minlength=N) + 1.0).astype(np.float64)
    dinv = (1.0 / np.sqrt(deg)).astype(np.float32)

    def conv(x, W, b):
        h = x @ W
        out = np.zeros_like(h)
        np.add.at(out, dst, h[src] * (dinv[src] * dinv[dst])[:, None])
        out += h * (dinv * dinv)[:, None]
        return out + b

    h = conv(z, W1, b1)
    h = conv(h, W2, b2)
    val = np.einsum('ef,ef->e', h[src], h[dst]).astype(np.float64)
    return (1.0 / (1.0 + np.exp(-val))).astype(np.float32)


def _plan(z, edge_index):
    import ml_dtypes
    bf16 = ml_dtypes.bfloat16
    N = z.shape[0]
    E = edge_index.shape[1]
    assert N % NC == 0
    npc = N // NC
    npad = ((npc + P - 1) // P) * P
    tiles = npad // P
    nrows = NC * npad
    nbuck = (nrows + BUCK - 1) // BUCK

    src = edge_index[0].astype(np.int64)
    dst = edge_index[1].astype(np.int64)
    deg = np.bincount(dst, minlength=N).astype(np.float64) + 1.0
    dinv = (1.0 / np.sqrt(deg)).astype(np.float32)

    owner_s, local_s = src // npc, src % npc
    owner_d, local_d = dst // npc, dst % npc
    pid_s = (owner_s * npad + local_s).astype(np.int64)
    b_s = (pid_s // BUCK).astype(np.int64)
    t_d = local_d // P
    dstloc = local_d % P

    plan = {
        'N': N, 'E': E, 'npc': npc, 'npad': npad, 'tiles': tiles,
        'nrows': nrows, 'nbuck': nbuck, 'dinv': dinv,
    }

    # ---------------- slot layout (shared by L1/L2/score) -----------------
    cell = (b_s * tiles + t_d).astype(np.int64)   # bucket-major cell id
    ncell = nbuck * tiles
    counts = np.zeros((NC, ncell), np.int64)
    for c in range(NC):
        counts[c] = np.bincount(cell[owner_d == c], minlength=ncell)
    K = np.ceil(counts.max(axis=0) / P).astype(np.int64)
    cell_ofs = np.concatenate([[0], np.cumsum(K * P)])
    S = int(cell_ofs[-1])
    totc = S // P
    totc_pad = ((totc + P - 1) // P) * P
    plan['K'] = K
    plan['cell_ofs'] = cell_ofs
    plan['S'] = S
    plan['totc_pad'] = totc_pad

    agg_idx = np.zeros((NC, 128, S // 16), np.int16)
    adl_w = np.zeros((NC, 128, S // P), bf16)
    adl_flat = np.zeros((NC, 1, S), bf16)
    out_core = owner_d
    out_slot = np.zeros(E, np.int64)
    for c in range(NC):
        m = owner_d == c
        cl = cell[m]
        order = np.argsort(cl, kind='stable')
        cl_s = cl[order]
        grp_start = np.searchsorted(cl_s, np.arange(ncell))
        rank = np.arange(cl_s.shape[0]) - grp_start[cl_s]
        slot = cell_ofs[cl_s] + rank
        eidx = np.nonzero(m)[0][order]
        out_slot[eidx] = slot
        idx_lin = np.zeros(S, np.int16)
        dl_lin = np.full(S, -1.0, np.float32)
        idx_lin[slot] = (pid_s[eidx] % BUCK).astype(np.int16)
        dl_lin[slot] = dstloc[eidx].astype(np.float32)
        agg_idx[c] = _wrap_idx16(idx_lin)
        adl_w[c] = np.ascontiguousarray(dl_lin.reshape(-1, P).T).astype(bf16)
        adl_flat[c, 0] = dl_lin.astype(bf16)
    plan['agg_idx'] = agg_idx
    plan['adl_w'] = adl_w
    plan['adl_flat'] = adl_flat
    plan['out_core'] = out_core
    plan['out_slot'] = out_slot

    # gather blocks: contiguous slot ranges within one src bucket
    blocks = []  # (bucket, slot_start, n_idxs)
    for b in range(nbuck):
        s0 = int(cell_ofs[b * tiles])
        s1 = int(cell_ofs[(b + 1) * tiles])
        s = s0
        while s < s1:
            n = min(BLK, s1 - s)
            blocks.append((b, s, n))
            s += n
    plan['blocks'] = blocks

    # ---------------- per-core node data ----------------------------------
    z_cols = np.zeros((NC, 128, tiles * 16), np.float32)
    dinv_cols = np.zeros((NC, 128, tiles), np.float32)
    for c in range(NC):
        zc = np.zeros((npad, 16), np.float32)
        zc[:npc] = z[c * npc:(c + 1) * npc]
        dc = np.zeros(npad, np.float32)
        dc[:npc] = dinv[c * npc:(c + 1) * npc]
        z_cols[c] = zc.reshape(tiles, P, 16).transpose(1, 0, 2).reshape(P, tiles * 16)
        dinv_cols[c] = dc.reshape(tiles, P).T
    plan['z_cols'] = z_cols
    plan['dinv_cols'] = dinv_cols
    plan['dinv2_cols'] = dinv_cols * dinv_cols
    return plan


def _build(plan, W1np, W2np):
    from concourse import bass, bacc, tile, mybir
    from concourse.masks import make_identity

    npad, tiles, nrows, nbuck = plan['npad'], plan['tiles'], plan['nrows'], plan['nbuck']
    S, totc_pad = plan['S'], plan['totc_pad']
    K, cell_ofs = plan['K'], plan['cell_ofs']
    blocks = plan['blocks']
    f32 = mybir.dt.float32
    bf16 = mybir.dt.bfloat16
    i16 = mybir.dt.int16
    AF = mybir.ActivationFunctionType
    ALU = mybir.AluOpType

    nc = bacc.Bacc("TRN2", target_bir_lowering=False, debug=False,
                   num_devices=NC, num_swdge_queues=4)

    # ---- I/O ----
    in_z = nc.dram_tensor("z_cols", [128, tiles * 16], f32, kind="ExternalInput")
    in_dinv = nc.dram_tensor("dinv_cols", [128, tiles], f32, kind="ExternalInput")
    in_dinv2 = nc.dram_tensor("dinv2_cols", [128, tiles], f32, kind="ExternalInput")
    in_w1t = nc.dram_tensor("w1t", [256, 16], f32, kind="ExternalInput")
    in_w2 = nc.dram_tensor("w2", [256, 256], f32, kind="ExternalInput")
    in_idx = nc.dram_tensor("agg_idx", [128, S // 16], i16, kind="ExternalInput")
    in_adl = nc.dram_tensor("adl_w", [128, S // P], bf16, kind="ExternalInput")
    in_adlf = nc.dram_tensor("adl_flat", [1, S], bf16, kind="ExternalInput")
    in_iota = nc.dram_tensor("iota_row", [128, 128], bf16, kind="ExternalInput")
    in_piota = nc.dram_tensor("piota_col", [128, 1], f32, kind="ExternalInput")
    in_ones = nc.dram_tensor("ones_row", [1, 128], bf16, kind="ExternalInput")
    out_val = nc.dram_tensor("out_val", [totc_pad, 128], f32, kind="ExternalOutput")

    qctr = [0]

    def next_q():
        q = qctr[0] % 4
        qctr[0] += 1
        return q

    with tile.TileContext(nc) as tc:
        with tc.tile_pool(name="res", bufs=1) as res, \
             tc.tile_pool(name="gat", bufs=10) as gat, \
             tc.tile_pool(name="idx", bufs=8) as idxp, \
             tc.tile_pool(name="oh", bufs=8) as ohp, \
             tc.tile_pool(name="ohx", bufs=12) as ohxp, \
             tc.tile_pool(name="sm", bufs=3) as sm, \
             tc.tile_pool(name="pagg", bufs=4, space="PSUM") as pagg, \
             tc.tile_pool(name="ptp", bufs=4, space="PSUM") as ptp, \
             tc.tile_pool(name="dram", bufs=1, space="DRAM") as dram:

            # ================= phase 0: constants, t0, G ==================
            ident = res.tile([128, 128], f32)
            make_identity(nc, ident[:])
            iota = res.tile([128, 128], bf16)
            nc.sync.dma_start(iota[:], in_iota[:])
            piota = res.tile([128, 1], f32)
            nc.sync.dma_start(piota[:], in_piota[:])
            ones_r = res.tile([1, 128], bf16)
            nc.sync.dma_start(ones_r[:], in_ones[:])

            dinv_t = res.tile([128, tiles], f32)
            nc.sync.dma_start(dinv_t[:], in_dinv[:])
            dinv2_t = res.tile([128, tiles], f32)
            nc.sync.dma_start(dinv2_t[:], in_dinv2[:])

            adl_t = res.tile([128, S // P], bf16)
            nc.sync.dma_start(adl_t[:], in_adl[:])

            # t0 = dinv * z ; s1 = dinv^2 * t0 (pre-scaled self-loop for L1)
            t0_c = res.tile([128, tiles * 16], f32)
            nc.sync.dma_start(t0_c[:], in_z[:])
            nc.vector.tensor_tensor(
                out=t0_c[:].rearrange("p (t f) -> p t f", f=16),
                in0=t0_c[:].rearrange("p (t f) -> p t f", f=16),
                in1=dinv_t[:][:, :, None].to_broadcast([128, tiles, 16]),
                op=ALU.mult)
            s1_c = res.tile([128, tiles * 16], f32)
            nc.vector.tensor_tensor(
                out=s1_c[:].rearrange("p (t f) -> p t f", f=16),
                in0=t0_c[:].rearrange("p (t f) -> p t f", f=16),
                in1=dinv2_t[:][:, :, None].to_broadcast([128, tiles, 16]),
                op=ALU.mult)

            # G = (W1 @ W2) @ (W1 @ W2)^T  [16,16] f32
            w1t_s = res.tile([128, 2 * 16], f32)
            nc.sync.dma_start(w1t_s[:, 0:16], in_w1t[0:128, :])
            nc.sync.dma_start(w1t_s[:, 16:32], in_w1t[128:256, :])
            w2_s = res.tile([128, 2 * 256], f32)
            nc.sync.dma_start(w2_s[:, 0:256], in_w2[0:128, :])
            nc.sync.dma_start(w2_s[:, 256:512], in_w2[128:256, :])
            w12_ps = ptp.tile([16, 256], f32, tag="tp", space="PSUM")
            nc.tensor.matmul(w12_ps[:], lhsT=w1t_s[:, 0:16], rhs=w2_s[:, 0:256], start=True, stop=False)
            nc.tensor.matmul(w12_ps[:], lhsT=w1t_s[:, 16:32], rhs=w2_s[:, 256:512], start=False, stop=True)
            w12_s = res.tile([16, 256], f32)
            nc.vector.tensor_copy(w12_s[:], w12_ps[:])
            w12T_s = res.tile([128, 2 * 16], f32)
            for blkk in range(2):
                tp = ptp.tile([128, 16], f32, tag="tp", space="PSUM")
                nc.tensor.transpose(tp[:], in_=w12_s[:, blkk * 128:(blkk + 1) * 128], identity=ident[:16, :16])
                nc.vector.tensor_copy(w12T_s[:, blkk * 16:(blkk + 1) * 16], tp[:])
            g_ps = ptp.tile([16, 16], f32, tag="tp", space="PSUM")
            nc.tensor.matmul(g_ps[:], lhsT=w12T_s[:, 0:16], rhs=w12T_s[:, 0:16], start=True, stop=False)
            nc.tensor.matmul(g_ps[:], lhsT=w12T_s[:, 16:32], rhs=w12T_s[:, 16:32], start=False, stop=True)
            g_s = res.tile([16, 16], f32)
            nc.vector.tensor_copy(g_s[:], g_ps[:])

            # ---- table write helper (cols f32 -> bf16 rows 0:16 of bounce) ----
            def table_write(cols_f32, bounce):
                h = sm.tile([128, tiles * 16], bf16, tag="casth")
                nc.vector.tensor_copy(h[:], cols_f32[:])
                dst = bounce[:].rearrange("(t p) f -> p t f", p=128)[:, :, 0:16]
                nc.sync.dma_start(dst, h[:].rearrange("p (t f) -> p t f", f=16))

            rg = [list(range(NC))]
            t0b = dram.tile([npad, FWH], bf16)
            t0f = dram.tile([nrows, FWH], bf16)
            table_write(t0_c, t0b)
            nc.gpsimd.collective_compute(
                "AllGather", ALU.bypass,
                ins=[t0b.opt()], outs=[t0f.opt()], replica_groups=rg)
            gen_oh_l1 = None  # created below once agg helpers are defined

            # ---- lazy gather-block machinery (per phase) -----------------
            bstarts = [b[1] for b in blocks]

            def make_get_block(table_full):
                # t-major consumption touches all 4 buckets in lockstep;
                # cache <=2 live blocks per bucket so the pool (bufs=10)
                # never deadlocks on slot reuse (consumption is monotonic
                # per bucket, so evicted blocks are never re-requested).
                blk_tiles = {}
                per_bucket = {b: [] for b in range(nbuck)}

                def get_block(slot):
                    bi = bisect.bisect_right(bstarts, slot) - 1
                    if bi in blk_tiles:
                        return blk_tiles[bi]
                    b, s0, n = blocks[bi]
                    it = idxp.tile([128, BLK // 16], i16, tag="idx")
                    nc.sync.dma_start(it[:, :n // 16], in_idx[:, s0 // 16:(s0 + n) // 16])
                    gt = gat.tile([128, (BLK // 128) * FWH], bf16, tag="gtab")
                    lo = b * BUCK
                    hi = min(lo + BUCK, nrows)
                    nc.gpsimd.dma_gather(
                        out_ap=gt[:, :(n // 128) * FWH].rearrange("p (c f) -> p c f", f=FWH),
                        in_ap=table_full[lo:hi, :],
                        idxs_ap=it[:, :n // 16],
                        num_idxs=n, num_idxs_reg=n, elem_size=FWH,
                        single_packet=False, queue_num=next_q())
                    blk_tiles[bi] = (gt, s0)
                    per_bucket[b].append(bi)
                    if len(per_bucket[b]) > 2:
                        del blk_tiles[per_bucket[b].pop(0)]
                    return blk_tiles[bi]
                return get_block

            # ================= aggregation layer ==========================
            def make_gen_oh():
                # cell one-hots, cached FIFO-6 (creation order == consumption
                # order), so a few cells can be pre-generated to overlap the
                # preceding AllGather.
                cache = {}
                fifo = []

                def gen_oh(cidx):
                    if cidx in cache:
                        return cache[cidx]
                    kk = int(K[cidx])
                    c0 = int(cell_ofs[cidx])
                    oh = ohp.tile([128, kk * 128], bf16, tag="oh")
                    nc.vector.tensor_tensor(
                        out=oh[:].rearrange("p (k q) -> p k q", q=128),
                        in0=iota[:][:, None, :].to_broadcast([128, kk, 128]),
                        in1=adl_t[:, c0 // P:c0 // P + kk][:, :, None]
                            .to_broadcast([128, kk, 128]),
                        op=ALU.is_equal)
                    cache[cidx] = oh
                    fifo.append(cidx)
                    if len(fifo) > 6:
                        del cache[fifo.pop(0)]
                    return oh
                return gen_oh

            def prewarm_oh(gen_oh, n=6):
                done = 0
                for t in range(tiles):
                    for b in range(nbuck):
                        cidx = b * tiles + t
                        if int(K[cidx]) == 0:
                            continue
                        gen_oh(cidx)
                        done += 1
                        if done >= n:
                            return

            def agg_layer(table_full, selfloop_sc, scale_t, out_c, gen_oh):
                get_block = make_get_block(table_full)
                for t in range(tiles):
                    # collect (cell, j) list for this tile
                    mms = []
                    for b in range(nbuck):
                        cidx = b * tiles + t
                        kk = int(K[cidx])
                        if kk == 0:
                            continue
                        mms.append((cidx, kk))
                    pt = pagg.tile([16, 128], f32, tag="agg", space="PSUM")
                    total = sum(kk for _, kk in mms)
                    done = 0
                    for cidx, kk in mms:
                        c0 = int(cell_ofs[cidx])
                        oh = gen_oh(cidx)
                        for j in range(kk):
                            slot = c0 + j * 128
                            gt, s0 = get_block(slot)
                            ch = (slot - s0) // 128
                            nc.tensor.matmul(
                                pt[:],
                                lhsT=gt[:].rearrange("p (c f) -> p c f", f=FWH)[:, ch, 0:16],
                                rhs=oh[:, j * 128:(j + 1) * 128],
                                start=(done == 0), stop=(done == total - 1))
                            done += 1
                    # epilogue: out[:, t] = scale * aggP + selfloop_scaled
                    aggT = sm.tile([16, 128], f32, tag="aggT")
                    nc.scalar.activation(aggT[:], pt[:], AF.Copy)
                    tpp = ptp.tile([128, 16], f32, tag="tp", space="PSUM")
                    nc.tensor.transpose(tpp[:], in_=aggT[:], identity=ident[:16, :16])
                    nc.vector.scalar_tensor_tensor(
                        out=out_c[:, t * 16:(t + 1) * 16],
                        in0=tpp[:],
                        scalar=scale_t[:, t:t + 1],
                        in1=selfloop_sc[:, t * 16:(t + 1) * 16],
                        op0=ALU.mult, op1=ALU.add)

            # L1: t1 = dinv2*agg1 + s1
            gen_oh_l1 = make_gen_oh()
            prewarm_oh(gen_oh_l1)
            t1_c = res.tile([128, tiles * 16], f32)
            agg_layer(t0f, s1_c, dinv2_t, t1_c, gen_oh_l1)
            # L2 one-hot prewarm overlaps the T1 AllGather below
            gen_oh_l2 = make_gen_oh()
            t1b = dram.tile([npad, FWH], bf16)
            t1f = dram.tile([nrows, FWH], bf16)
            table_write(t1_c, t1b)
            prewarm_oh(gen_oh_l2)
            nc.gpsimd.collective_compute(
                "AllGather", ALU.bypass,
                ins=[t1b.opt()], outs=[t1f.opt()], replica_groups=rg)

            # L2: u = dinv*agg2 + dinv*t1
            s2_c = res.tile([128, tiles * 16], f32)
            nc.vector.tensor_tensor(
                out=s2_c[:].rearrange("p (t f) -> p t f", f=16),
                in0=t1_c[:].rearrange("p (t f) -> p t f", f=16),
                in1=dinv_t[:][:, :, None].to_broadcast([128, tiles, 16]),
                op=ALU.mult)
            u_c = res.tile([128, tiles * 16], f32)
            agg_layer(t1f, s2_c, dinv_t, u_c, gen_oh_l2)

            ub = dram.tile([npad, FWH], bf16)
            uf = dram.tile([nrows, FWH], bf16)
            table_write(u_c, ub)

            # w = u @ G per tile (bf16 cols for the uexp matmuls)
            w_c = res.tile([128, tiles * 16], bf16)
            for t in range(tiles):
                tpu = ptp.tile([16, 128], f32, tag="tp", space="PSUM")
                nc.tensor.transpose(tpu[:], in_=u_c[:, t * 16:(t + 1) * 16], identity=ident[:])
                uT = sm.tile([16, 128], f32, tag="uT")
                nc.scalar.activation(uT[:], tpu[:], AF.Copy)
                wp = ptp.tile([128, 16], f32, tag="tp", space="PSUM")
                nc.tensor.matmul(wp[:], lhsT=uT[:], rhs=g_s[:], start=True, stop=True)
                nc.scalar.activation(w_c[:, t * 16:(t + 1) * 16], wp[:], AF.Copy)

            # ================= scoring ====================================
            # OH_exp segments (lazy, 512 slots each):
            # replica = ones^T @ adl_flat (PE) -> is_equal(piota) off PSUM
            SEG = 512
            seg_tiles = {}
            seg_fifo = []

            def get_seg(slot):
                si = slot // SEG
                if si in seg_tiles:
                    return seg_tiles[si]
                s0 = si * SEG
                n = min(SEG, S - s0)
                adlf = idxp.tile([1, SEG], bf16, tag="adlf")
                nc.sync.dma_start(adlf[:, :n], in_adlf[:, s0:s0 + n])
                rep_ps = pagg.tile([128, SEG], f32, tag="agg", space="PSUM")
                nc.tensor.matmul(rep_ps[:, :n], lhsT=ones_r[:], rhs=adlf[:, :n],
                                 start=True, stop=True)
                ohx = ohxp.tile([128, SEG], bf16, tag="ohx")
                nc.vector.tensor_scalar(
                    out=ohx[:, :n],
                    in0=rep_ps[:, :n],
                    scalar1=piota[:, 0:1],
                    scalar2=None,
                    op0=ALU.is_equal)
                seg_tiles[si] = (ohx, s0)
                seg_fifo.append(si)
                if len(seg_fifo) > 12:
                    del seg_tiles[seg_fifo.pop(0)]
                return seg_tiles[si]

            # prewarm OH_exp segments in consumption order, then issue the
            # U AllGather so the segment chain overlaps the collective
            seen = []
            for t in range(tiles):
                for b in range(nbuck):
                    cidx = b * tiles + t
                    kk = int(K[cidx])
                    c0 = int(cell_ofs[cidx])
                    for j in range(kk):
                        si = (c0 + j * 128) // SEG
                        if si not in seen:
                            seen.append(si)
                if len(seen) >= 8:
                    break
            for si in seen[:8]:
                get_seg(si * SEG)
            nc.gpsimd.collective_compute(
                "AllGather", ALU.bypass,
                ins=[ub.opt()], outs=[uf.opt()], replica_groups=rg)

            get_score_block = make_get_block(uf)
            val = res.tile([128, totc_pad], f32)
            nc.vector.memset(val[:], 0.0)
            for t in range(tiles):
                for b in range(nbuck):
                    cidx = b * tiles + t
                    kk = int(K[cidx])
                    if kk == 0:
                        continue
                    c0 = int(cell_ofs[cidx])
                    uex = ptp.tile([128, kk * 16], f32, tag="tp", space="PSUM")
                    for j in range(kk):
                        slot = c0 + j * 128
                        ohx, s0 = get_seg(slot)
                        off = slot - s0
                        nc.tensor.matmul(
                            uex[:, j * 16:(j + 1) * 16],
                            lhsT=ohx[:, off:off + 128],
                            rhs=w_c[:, t * 16:(t + 1) * 16],
                            start=True, stop=True)
                    # dot with gathered u[src] rows, split on block straddle
                    j = 0
                    while j < kk:
                        slot = c0 + j * 128
                        gt, s0 = get_score_block(slot)
                        ch = (slot - s0) // 128
                        # how many chunks stay inside this block?
                        room = (s0 + BLK - slot) // 128
                        m = min(kk - j, room)
                        prod = sm.tile([128, kk * 16], f32, tag="prod")
                        nc.vector.tensor_tensor(
                            out=prod[:, j * 16:(j + m) * 16].rearrange("p (c f) -> p c f", f=16),
                            in0=uex[:, j * 16:(j + m) * 16].rearrange("p (c f) -> p c f", f=16),
                            in1=gt[:].rearrange("p (c f) -> p c f", f=FWH)[:, ch:ch + m, 0:16],
                            op=ALU.mult)
                        nc.vector.reduce_sum(
                            out=val[:, c0 // P + j:c0 // P + j + m],
                            in_=prod[:, j * 16:(j + m) * 16].rearrange("p (c f) -> p c f", f=16),
                            axis=mybir.AxisListType.X)
                        j += m

            # sigmoid + transpose + out
            for g in range(totc_pad // 128):
                sg = sm.tile([128, 128], f32, tag="sig")
                nc.scalar.activation(sg[:], val[:, g * 128:(g + 1) * 128], AF.Sigmoid)
                tp = ptp.tile([128, 128], f32, tag="tp", space="PSUM")
                nc.tensor.transpose(tp[:], in_=sg[:], identity=ident[:])
                so = sm.tile([128, 128], f32, tag="sigT")
                nc.vector.tensor_copy(so[:], tp[:])
                nc.sync.dma_start(out_val[g * 128:(g + 1) * 128, :], so[:])

    nc.compile()
    return nc


_CACHE = {}


def kernel(z, edge_index, W1, b1, W2, b2):
    z = np.asarray(z, np.float32)
    edge_index = np.asarray(edge_index)
    W1 = np.asarray(W1, np.float32)
    W2 = np.asarray(W2, np.float32)
    b1 = np.asarray(b1, np.float32)
    b2 = np.asarray(b2, np.float32)
    if np.any(b1 != 0) or np.any(b2 != 0):
        return _host_reference(z, edge_index, W1, b1, W2, b2)

    import ml_dtypes
    from concourse import bass_utils
    bf16 = ml_dtypes.bfloat16

    plan = _plan(z, edge_index)
    key = (z.shape, edge_index.shape, plan['S'], tuple(plan['K'].tolist()))
    if key not in _CACHE:
        _CACHE.clear()
        _CACHE[key] = _build(plan, W1, W2)
    nc = _CACHE[key]

    w1t = np.ascontiguousarray(W1.T)
    iota_row = np.ascontiguousarray(
        np.tile(np.arange(128, dtype=np.float32), (128, 1))).astype(bf16)
    piota_col = np.arange(128, dtype=np.float32).reshape(128, 1)
    ones_row = np.ones((1, 128), np.float32).astype(bf16)
    in_maps = []
    for c in range(NC):
        in_maps.append({
            "z_cols": plan['z_cols'][c],
            "dinv_cols": plan['dinv_cols'][c],
            "dinv2_cols": plan['dinv2_cols'][c],
            "w1t": w1t, "w2": W2,
            "agg_idx": plan['agg_idx'][c],
            "adl_w": plan['adl_w'][c],
            "adl_flat": plan['adl_flat'][c],
            "iota_row": iota_row,
            "piota_col": piota_col,
            "ones_row": ones_row,
        })
    res = bass_utils.run_bass_kernel_spmd(nc, in_maps, core_ids=list(range(NC)))
    kernel._last = (nc, in_maps, plan)

    E = plan['E']
    flats = [res.results[c]["out_val"].reshape(-1) for c in range(NC)]
    out = np.empty(E, np.float32)
    oc, osl = plan['out_core'], plan['out_slot']
    for c in range(NC):
        m = oc == c
        out[m] = flats[c][osl[m]]
    return out


# revision 3
# speedup vs baseline: 1.1618x; 1.0215x over previous
"""GCN link-decoder kernel for 8 TRN2 NeuronCores — v2 (SWDGE-queue parallel).

Math (both GCNConv layers are linear, b1=b2=0): with P = D^-1/2 (A+I) D^-1/2,
    t0 = dinv*z; agg1 = scatter-sum t0[src]; t1 = dinv^2*(agg1 + t0)
    agg2 = scatter-sum t1[src]; u = dinv*(agg2 + t1); w = u @ G, G=(W1W2)(W1W2)^T
    val_e = u[src_e] . w[dst_e];  out = sigmoid(val)

Design notes (from the profiled baseline, 17.0 ms):
- The bottleneck was SWDGE descriptor generation on the GpSimd engine
  (~7.8 ns/gather-index, 13.1 ms of Pool time). v2 cuts per-edge gather
  streams from 4 to 3 (L1, L2, score-u) by computing the dst-side score
  factor w[dst_e] with a one-hot expansion matmul on TensorE, and spreads
  the remaining gathers across all 4 SWDGE queues, whose descriptor
  generation runs concurrently (measured 3-4x).
- Everything per-edge is processed in ONE slot layout shared by all three
  phases: cell = (src bucket, dst tile) at the dst-owner core, bucket-major
  slots, t-major processing (PSUM accumulates the 4 buckets per dst tile).
- Tables are bf16 (logits are in [-0.5, 0.7], so bf16 end-to-end error is
  ~1e-3 abs, far inside the 2e-2 gate); one-hots bf16 (2x DVE is_equal,
  FWL weight loads).
"""
import sys
import os
import bisect
sys.path.insert(0, '/opt/trn_rl_repo')
import numpy as np

NC = 8          # cores
P = 128         # partitions
FWH = 128       # table row width in bf16 elems (256B dma_gather granule)
BUCK = 32768    # int16 index bucket size (table rows per bucket)
BLK = 4096      # gather idxs per dma_gather instruction


def _wrap_idx16(arr: np.ndarray) -> np.ndarray:
    """Linear int16 slot-index array (len % 128 == 0) -> [128, len/16] SWDGE
    wrapped layout (slot k at partition k%16, col k//16; replicated to 128)."""
    n = arr.shape[0]
    t16 = arr.reshape(n // 16, 16).T
    return np.ascontiguousarray(np.tile(t16, (8, 1)))


def _host_reference(z, edge_index, W1, b1, W2, b2):
    """Numpy fallback (used only when b1/b2 are nonzero)."""
    N = z.shape[0]
    src, dst = edge_index[0], edge_index[1]
    deg = (np.bincount(dst, minlength=N) + 1.0).astype(np.float64)
    dinv = (1.0 / np.sqrt(deg)).astype(np.float32)

    def conv(x, W, b):
        h = x @ W
        out = np.zeros_like(h)
        np.add.at(out, dst, h[src] * (dinv[src] * dinv[dst])[:, None])
        out += h * (dinv * dinv)[:, None]
        return out + b

    h = conv(z, W1, b1)
    h = conv(h, W2, b2)
    val = np.einsum('ef,ef->e', h[src], h[dst]).astype(np.float64)
    return (1.0 / (1.0 + np.exp(-val))).astype(np.float32)


def _plan(z, edge_index):
    import ml_dtypes
    bf16 = ml_dtypes.bfloat16
    N = z.shape[0]
    E = edge_index.shape[1]
    assert N % NC == 0
    npc = N // NC
    npad = ((npc + P - 1) // P) * P
    tiles = npad // P
    nrows = NC * npad
    nbuck = (nrows + BUCK - 1) // BUCK

    src = edge_index[0].astype(np.int64)
    dst = edge_index[1].astype(np.int64)
    deg = np.bincount(dst, minlength=N).astype(np.float64) + 1.0
    dinv = (1.0 / np.sqrt(deg)).astype(np.float32)

    owner_s, local_s = src // npc, src % npc
    owner_d, local_d = dst // npc, dst % npc
    pid_s = (owner_s * npad + local_s).astype(np.int64)
    b_s = (pid_s // BUCK).astype(np.int64)
    t_d = local_d // P
    dstloc = local_d % P

    plan = {
        'N': N, 'E': E, 'npc': npc, 'npad': npad, 'tiles': tiles,
        'nrows': nrows, 'nbuck': nbuck, 'dinv': dinv,
    }

    # ---------------- slot layout (shared by L1/L2/score) -----------------
    cell = (b_s * tiles + t_d).astype(np.int64)   # bucket-major cell id
    ncell = nbuck * tiles
    counts = np.zeros((NC, ncell), np.int64)
    for c in range(NC):
        counts[c] = np.bincount(cell[owner_d == c], minlength=ncell)
    K = np.ceil(counts.max(axis=0) / P).astype(np.int64)
    cell_ofs = np.concatenate([[0], np.cumsum(K * P)])
    S = int(cell_ofs[-1])
    totc = S // P
    totc_pad = ((totc + P - 1) // P) * P
    plan['K'] = K
    plan['cell_ofs'] = cell_ofs
    plan['S'] = S
    plan['totc_pad'] = totc_pad

    agg_idx = np.zeros((NC, 128, S // 16), np.int16)
    adl_w = np.zeros((NC, 128, S // P), bf16)
    adl_flat = np.zeros((NC, 1, S), bf16)
    out_core = owner_d
    out_slot = np.zeros(E, np.int64)
    for c in range(NC):
        m = owner_d == c
        cl = cell[m]
        order = np.argsort(cl, kind='stable')
        cl_s = cl[order]
        grp_start = np.searchsorted(cl_s, np.arange(ncell))
        rank = np.arange(cl_s.shape[0]) - grp_start[cl_s]
        slot = cell_ofs[cl_s] + rank
        eidx = np.nonzero(m)[0][order]
        out_slot[eidx] = slot
        idx_lin = np.zeros(S, np.int16)
        dl_lin = np.full(S, -1.0, np.float32)
        idx_lin[slot] = (pid_s[eidx] % BUCK).astype(np.int16)
        dl_lin[slot] = dstloc[eidx].astype(np.float32)
        agg_idx[c] = _wrap_idx16(idx_lin)
        adl_w[c] = np.ascontiguousarray(dl_lin.reshape(-1, P).T).astype(bf16)
        adl_flat[c, 0] = dl_lin.astype(bf16)
    plan['agg_idx'] = agg_idx
    plan['adl_w'] = adl_w
    plan['adl_flat'] = adl_flat
    plan['out_core'] = out_core
    plan['out_slot'] = out_slot

    # gather blocks: contiguous slot ranges within one src bucket
    blocks = []  # (bucket, slot_start, n_idxs)
    for b in range(nbuck):
        s0 = int(cell_ofs[b * tiles])
        s1 = int(cell_ofs[(b + 1) * tiles])
        s = s0
        while s < s1:
            n = min(BLK, s1 - s)
            blocks.append((b, s, n))
            s += n
    plan['blocks'] = blocks

    # ---------------- per-core node data ----------------------------------
    z_cols = np.zeros((NC, 128, tiles * 16), np.float32)
    dinv_cols = np.zeros((NC, 128, tiles), np.float32)
    for c in range(NC):
        zc = np.zeros((npad, 16), np.float32)
        zc[:npc] = z[c * npc:(c + 1) * npc]
        dc = np.zeros(npad, np.float32)
        dc[:npc] = dinv[c * npc:(c + 1) * npc]
        z_cols[c] = zc.reshape(tiles, P, 16).transpose(1, 0, 2).reshape(P, tiles * 16)
        dinv_cols[c] = dc.reshape(tiles, P).T
    plan['z_cols'] = z_cols
    plan['dinv_cols'] = dinv_cols
    plan['dinv2_cols'] = dinv_cols * dinv_cols
    return plan


def _build(plan, W1np, W2np):
    from concourse import bass, bacc, tile, mybir
    from concourse.masks import make_identity

    npad, tiles, nrows, nbuck = plan['npad'], plan['tiles'], plan['nrows'], plan['nbuck']
    S, totc_pad = plan['S'], plan['totc_pad']
    K, cell_ofs = plan['K'], plan['cell_ofs']
    blocks = plan['blocks']
    f32 = mybir.dt.float32
    bf16 = mybir.dt.bfloat16
    i16 = mybir.dt.int16
    AF = mybir.ActivationFunctionType
    ALU = mybir.AluOpType

    nc = bacc.Bacc("TRN2", target_bir_lowering=False, debug=False,
                   num_devices=NC, num_swdge_queues=4)

    # ---- I/O ----
    in_z = nc.dram_tensor("z_cols", [128, tiles * 16], f32, kind="ExternalInput")
    in_dinv = nc.dram_tensor("dinv_cols", [128, tiles], f32, kind="ExternalInput")
    in_dinv2 = nc.dram_tensor("dinv2_cols", [128, tiles], f32, kind="ExternalInput")
    in_w1t = nc.dram_tensor("w1t", [256, 16], f32, kind="ExternalInput")
    in_w2 = nc.dram_tensor("w2", [256, 256], f32, kind="ExternalInput")
    in_idx = nc.dram_tensor("agg_idx", [128, S // 16], i16, kind="ExternalInput")
    in_adl = nc.dram_tensor("adl_w", [128, S // P], bf16, kind="ExternalInput")
    in_adlf = nc.dram_tensor("adl_flat", [1, S], bf16, kind="ExternalInput")
    in_iota = nc.dram_tensor("iota_row", [128, 128], bf16, kind="ExternalInput")
    in_piota = nc.dram_tensor("piota_col", [128, 1], f32, kind="ExternalInput")
    in_ones = nc.dram_tensor("ones_row", [1, 128], bf16, kind="ExternalInput")
    out_val = nc.dram_tensor("out_val", [totc_pad, 128], f32, kind="ExternalOutput")

    qctr = [0]

    def next_q():
        q = qctr[0] % 4
        qctr[0] += 1
        return q

    with tile.TileContext(nc) as tc:
        with tc.tile_pool(name="res", bufs=1) as res, \
             tc.tile_pool(name="gat", bufs=10) as gat, \
             tc.tile_pool(name="idx", bufs=8) as idxp, \
             tc.tile_pool(name="oh", bufs=8) as ohp, \
             tc.tile_pool(name="ohx", bufs=12) as ohxp, \
             tc.tile_pool(name="sm", bufs=3) as sm, \
             tc.tile_pool(name="pagg", bufs=4, space="PSUM") as pagg, \
             tc.tile_pool(name="ptp", bufs=4, space="PSUM") as ptp, \
             tc.tile_pool(name="dram", bufs=1, space="DRAM") as dram:

            # ================= phase 0: constants, t0, G ==================
            ident = res.tile([128, 128], f32)
            make_identity(nc, ident[:])

            # hoist num_idxs registers: one MOVE each, reused by every
            # dma_gather (a fresh to_reg per gather creates a WAR chain on
            # the shared register that serializes gather dispatch)
            nregs = {}

            def nreg(n):
                if n not in nregs:
                    nregs[n] = nc.gpsimd.to_reg(n)
                return nregs[n]
            iota = res.tile([128, 128], bf16)
            nc.sync.dma_start(iota[:], in_iota[:])
            piota = res.tile([128, 1], f32)
            nc.sync.dma_start(piota[:], in_piota[:])
            ones_r = res.tile([1, 128], bf16)
            nc.sync.dma_start(ones_r[:], in_ones[:])

            dinv_t = res.tile([128, tiles], f32)
            nc.sync.dma_start(dinv_t[:], in_dinv[:])
            dinv2_t = res.tile([128, tiles], f32)
            nc.sync.dma_start(dinv2_t[:], in_dinv2[:])

            adl_t = res.tile([128, S // P], bf16)
            nc.sync.dma_start(adl_t[:], in_adl[:])

            # t0 = dinv * z ; s1 = dinv^2 * t0 (pre-scaled self-loop for L1)
            t0_c = res.tile([128, tiles * 16], f32)
            nc.sync.dma_start(t0_c[:], in_z[:])
            nc.vector.tensor_tensor(
                out=t0_c[:].rearrange("p (t f) -> p t f", f=16),
                in0=t0_c[:].rearrange("p (t f) -> p t f", f=16),
                in1=dinv_t[:][:, :, None].to_broadcast([128, tiles, 16]),
                op=ALU.mult)
            s1_c = res.tile([128, tiles * 16], f32)
            nc.vector.tensor_tensor(
                out=s1_c[:].rearrange("p (t f) -> p t f", f=16),
                in0=t0_c[:].rearrange("p (t f) -> p t f", f=16),
                in1=dinv2_t[:][:, :, None].to_broadcast([128, tiles, 16]),
                op=ALU.mult)

            # G = (W1 @ W2) @ (W1 @ W2)^T  [16,16] f32
            w1t_s = res.tile([128, 2 * 16], f32)
            nc.sync.dma_start(w1t_s[:, 0:16], in_w1t[0:128, :])
            nc.sync.dma_start(w1t_s[:, 16:32], in_w1t[128:256, :])
            w2_s = res.tile([128, 2 * 256], f32)
            nc.sync.dma_start(w2_s[:, 0:256], in_w2[0:128, :])
            nc.sync.dma_start(w2_s[:, 256:512], in_w2[128:256, :])
            w12_ps = ptp.tile([16, 256], f32, tag="tp", space="PSUM")
            nc.tensor.matmul(w12_ps[:], lhsT=w1t_s[:, 0:16], rhs=w2_s[:, 0:256], start=True, stop=False)
            nc.tensor.matmul(w12_ps[:], lhsT=w1t_s[:, 16:32], rhs=w2_s[:, 256:512], start=False, stop=True)
            w12_s = res.tile([16, 256], f32)
            nc.vector.tensor_copy(w12_s[:], w12_ps[:])
            w12T_s = res.tile([128, 2 * 16], f32)
            for blkk in range(2):
                tp = ptp.tile([128, 16], f32, tag="tp", space="PSUM")
                nc.tensor.transpose(tp[:], in_=w12_s[:, blkk * 128:(blkk + 1) * 128], identity=ident[:16, :16])
                nc.vector.tensor_copy(w12T_s[:, blkk * 16:(blkk + 1) * 16], tp[:])
            g_ps = ptp.tile([16, 16], f32, tag="tp", space="PSUM")
            nc.tensor.matmul(g_ps[:], lhsT=w12T_s[:, 0:16], rhs=w12T_s[:, 0:16], start=True, stop=False)
            nc.tensor.matmul(g_ps[:], lhsT=w12T_s[:, 16:32], rhs=w12T_s[:, 16:32], start=False, stop=True)
            g_s = res.tile([16, 16], f32)
            nc.vector.tensor_copy(g_s[:], g_ps[:])

            # ---- table write helper (cols f32 -> bf16 rows 0:16 of bounce) ----
            def table_write(cols_f32, bounce):
                h = sm.tile([128, tiles * 16], bf16, tag="casth")
                nc.vector.tensor_copy(h[:], cols_f32[:])
                dst = bounce[:].rearrange("(t p) f -> p t f", p=128)[:, :, 0:16]
                nc.sync.dma_start(dst, h[:].rearrange("p (t f) -> p t f", f=16))

            rg = [list(range(NC))]
            t0b = dram.tile([npad, FWH], bf16)
            t0f = dram.tile([nrows, FWH], bf16)
            table_write(t0_c, t0b)
            nc.gpsimd.collective_compute(
                "AllGather", ALU.bypass,
                ins=[t0b.opt()], outs=[t0f.opt()], replica_groups=rg)
            gen_oh_l1 = None  # created below once agg helpers are defined

            # ---- lazy gather-block machinery (per phase) -----------------
            bstarts = [b[1] for b in blocks]

            def make_get_block(table_full):
                # t-major consumption touches all 4 buckets in lockstep;
                # cache <=2 live blocks per bucket so the pool (bufs=10)
                # never deadlocks on slot reuse (consumption is monotonic
                # per bucket, so evicted blocks are never re-requested).
                blk_tiles = {}
                per_bucket = {b: [] for b in range(nbuck)}

                def get_block(slot):
                    bi = bisect.bisect_right(bstarts, slot) - 1
                    if bi in blk_tiles:
                        return blk_tiles[bi]
                    b, s0, n = blocks[bi]
                    it = idxp.tile([128, BLK // 16], i16, tag="idx")
                    nc.sync.dma_start(it[:, :n // 16], in_idx[:, s0 // 16:(s0 + n) // 16])
                    gt = gat.tile([128, (BLK // 128) * FWH], bf16, tag="gtab")
                    lo = b * BUCK
                    hi = min(lo + BUCK, nrows)
                    nc.gpsimd.dma_gather(
                        out_ap=gt[:, :(n // 128) * FWH].rearrange("p (c f) -> p c f", f=FWH),
                        in_ap=table_full[lo:hi, :],
                        idxs_ap=it[:, :n // 16],
                        num_idxs=n, num_idxs_reg=nreg(n), elem_size=FWH,
                        single_packet=False, queue_num=next_q())
                    blk_tiles[bi] = (gt, s0)
                    per_bucket[b].append(bi)
                    if len(per_bucket[b]) > 2:
                        del blk_tiles[per_bucket[b].pop(0)]
                    return blk_tiles[bi]
                return get_block

            # ================= aggregation layer ==========================
            def make_gen_oh():
                # cell one-hots, cached FIFO-6 (creation order == consumption
                # order), so a few cells can be pre-generated to overlap the
                # preceding AllGather.
                cache = {}
                fifo = []

                def gen_oh(cidx):
                    if cidx in cache:
                        return cache[cidx]
                    kk = int(K[cidx])
                    c0 = int(cell_ofs[cidx])
                    oh = ohp.tile([128, kk * 128], bf16, tag="oh")
                    nc.vector.tensor_tensor(
                        out=oh[:].rearrange("p (k q) -> p k q", q=128),
                        in0=iota[:][:, None, :].to_broadcast([128, kk, 128]),
                        in1=adl_t[:, c0 // P:c0 // P + kk][:, :, None]
                            .to_broadcast([128, kk, 128]),
                        op=ALU.is_equal)
                    cache[cidx] = oh
                    fifo.append(cidx)
                    if len(fifo) > 6:
                        del cache[fifo.pop(0)]
                    return oh
                return gen_oh

            def prewarm_oh(gen_oh, n=6):
                done = 0
                for t in range(tiles):
                    for b in range(nbuck):
                        cidx = b * tiles + t
                        if int(K[cidx]) == 0:
                            continue
                        gen_oh(cidx)
                        done += 1
                        if done >= n:
                            return

            def agg_layer(table_full, selfloop_sc, scale_t, out_c, gen_oh):
                get_block = make_get_block(table_full)
                for t in range(tiles):
                    # collect (cell, j) list for this tile
                    mms = []
                    for b in range(nbuck):
                        cidx = b * tiles + t
                        kk = int(K[cidx])
                        if kk == 0:
                            continue
                        mms.append((cidx, kk))
                    pt = pagg.tile([16, 128], f32, tag="agg", space="PSUM")
                    total = sum(kk for _, kk in mms)
                    done = 0
                    for cidx, kk in mms:
                        c0 = int(cell_ofs[cidx])
                        oh = gen_oh(cidx)
                        for j in range(kk):
                            slot = c0 + j * 128
                            gt, s0 = get_block(slot)
                            ch = (slot - s0) // 128
                            nc.tensor.matmul(
                                pt[:],
                                lhsT=gt[:].rearrange("p (c f) -> p c f", f=FWH)[:, ch, 0:16],
                                rhs=oh[:, j * 128:(j + 1) * 128],
                                start=(done == 0), stop=(done == total - 1))
                            done += 1
                    # epilogue: out[:, t] = scale * aggP + selfloop_scaled
                    aggT = sm.tile([16, 128], f32, tag="aggT")
                    nc.scalar.activation(aggT[:], pt[:], AF.Copy)
                    tpp = ptp.tile([128, 16], f32, tag="tp", space="PSUM")
                    nc.tensor.transpose(tpp[:], in_=aggT[:], identity=ident[:16, :16])
                    nc.vector.scalar_tensor_tensor(
                        out=out_c[:, t * 16:(t + 1) * 16],
                        in0=tpp[:],
                        scalar=scale_t[:, t:t + 1],
                        in1=selfloop_sc[:, t * 16:(t + 1) * 16],
                        op0=ALU.mult, op1=ALU.add)

            # L1: t1 = dinv2*agg1 + s1
            gen_oh_l1 = make_gen_oh()
            prewarm_oh(gen_oh_l1)
            t1_c = res.tile([128, tiles * 16], f32)
            agg_layer(t0f, s1_c, dinv2_t, t1_c, gen_oh_l1)
            # L2 one-hot prewarm overlaps the T1 AllGather below
            gen_oh_l2 = make_gen_oh()
            t1b = dram.tile([npad, FWH], bf16)
            t1f = dram.tile([nrows, FWH], bf16)
            table_write(t1_c, t1b)
            prewarm_oh(gen_oh_l2)
            nc.gpsimd.collective_compute(
                "AllGather", ALU.bypass,
                ins=[t1b.opt()], outs=[t1f.opt()], replica_groups=rg)

            # L2: u = dinv*agg2 + dinv*t1
            s2_c = res.tile([128, tiles * 16], f32)
            nc.vector.tensor_tensor(
                out=s2_c[:].rearrange("p (t f) -> p t f", f=16),
                in0=t1_c[:].rearrange("p (t f) -> p t f", f=16),
                in1=dinv_t[:][:, :, None].to_broadcast([128, tiles, 16]),
                op=ALU.mult)
            u_c = res.tile([128, tiles * 16], f32)
            agg_layer(t1f, s2_c, dinv_t, u_c, gen_oh_l2)

            ub = dram.tile([npad, FWH], bf16)
            uf = dram.tile([nrows, FWH], bf16)
            table_write(u_c, ub)

            # w = u @ G per tile (bf16 cols for the uexp matmuls)
            w_c = res.tile([128, tiles * 16], bf16)
            for t in range(tiles):
                tpu = ptp.tile([16, 128], f32, tag="tp", space="PSUM")
                nc.tensor.transpose(tpu[:], in_=u_c[:, t * 16:(t + 1) * 16], identity=ident[:])
                uT = sm.tile([16, 128], f32, tag="uT")
                nc.scalar.activation(uT[:], tpu[:], AF.Copy)
                wp = ptp.tile([128, 16], f32, tag="tp", space="PSUM")
                nc.tensor.matmul(wp[:], lhsT=uT[:], rhs=g_s[:], start=True, stop=True)
                nc.scalar.activation(w_c[:, t * 16:(t + 1) * 16], wp[:], AF.Copy)

            # ================= scoring ====================================
            # OH_exp segments (lazy, 512 slots each):
            # replica = ones^T @ adl_flat (PE) -> is_equal(piota) off PSUM
            SEG = 512
            seg_tiles = {}
            seg_fifo = []

            def get_seg(slot):
                si = slot // SEG
                if si in seg_tiles:
                    return seg_tiles[si]
                s0 = si * SEG
                n = min(SEG, S - s0)
                adlf = idxp.tile([1, SEG], bf16, tag="adlf")
                nc.sync.dma_start(adlf[:, :n], in_adlf[:, s0:s0 + n])
                rep_ps = pagg.tile([128, SEG], f32, tag="agg", space="PSUM")
                nc.tensor.matmul(rep_ps[:, :n], lhsT=ones_r[:], rhs=adlf[:, :n],
                                 start=True, stop=True)
                ohx = ohxp.tile([128, SEG], bf16, tag="ohx")
                nc.vector.tensor_scalar(
                    out=ohx[:, :n],
                    in0=rep_ps[:, :n],
                    scalar1=piota[:, 0:1],
                    scalar2=None,
                    op0=ALU.is_equal)
                seg_tiles[si] = (ohx, s0)
                seg_fifo.append(si)
                if len(seg_fifo) > 12:
                    del seg_tiles[seg_fifo.pop(0)]
                return seg_tiles[si]

            # prewarm OH_exp segments in consumption order, then issue the
            # U AllGather so the segment chain overlaps the collective
            seen = []
            for t in range(tiles):
                for b in range(nbuck):
                    cidx = b * tiles + t
                    kk = int(K[cidx])
                    c0 = int(cell_ofs[cidx])
                    for j in range(kk):
                        si = (c0 + j * 128) // SEG
                        if si not in seen:
                            seen.append(si)
                if len(seen) >= 8:
                    break
            for si in seen[:8]:
                get_seg(si * SEG)
            nc.gpsimd.collective_compute(
                "AllGather", ALU.bypass,
                ins=[ub.opt()], outs=[uf.opt()], replica_groups=rg)

            get_score_block = make_get_block(uf)
            val = res.tile([128, totc_pad], f32)
            nc.vector.memset(val[:], 0.0)
            for t in range(tiles):
                for b in range(nbuck):
                    cidx = b * tiles + t
                    kk = int(K[cidx])
                    if kk == 0:
                        continue
                    c0 = int(cell_ofs[cidx])
                    uex = ptp.tile([128, kk * 16], f32, tag="tp", space="PSUM")
                    for j in range(kk):
                        slot = c0 + j * 128
                        ohx, s0 = get_seg(slot)
                        off = slot - s0
                        nc.tensor.matmul(
                            uex[:, j * 16:(j + 1) * 16],
                            lhsT=ohx[:, off:off + 128],
                            rhs=w_c[:, t * 16:(t + 1) * 16],
                            start=True, stop=True)
                    # dot with gathered u[src] rows, split on block straddle
                    j = 0
                    while j < kk:
                        slot = c0 + j * 128
                        gt, s0 = get_score_block(slot)
                        ch = (slot - s0) // 128
                        # how many chunks stay inside this block?
                        room = (s0 + BLK - slot) // 128
                        m = min(kk - j, room)
                        prod = sm.tile([128, kk * 16], f32, tag="prod")
                        nc.vector.tensor_tensor(
                            out=prod[:, j * 16:(j + m) * 16].rearrange("p (c f) -> p c f", f=16),
                            in0=uex[:, j * 16:(j + m) * 16].rearrange("p (c f) -> p c f", f=16),
                            in1=gt[:].rearrange("p (c f) -> p c f", f=FWH)[:, ch:ch + m, 0:16],
                            op=ALU.mult)
                        nc.vector.reduce_sum(
                            out=val[:, c0 // P + j:c0 // P + j + m],
                            in_=prod[:, j * 16:(j + m) * 16].rearrange("p (c f) -> p c f", f=16),
                            axis=mybir.AxisListType.X)
                        j += m

            # sigmoid + transpose + out
            for g in range(totc_pad // 128):
                sg = sm.tile([128, 128], f32, tag="sig")
                nc.scalar.activation(sg[:], val[:, g * 128:(g + 1) * 128], AF.Sigmoid)
                tp = ptp.tile([128, 128], f32, tag="tp", space="PSUM")
                nc.tensor.transpose(tp[:], in_=sg[:], identity=ident[:])
                so = sm.tile([128, 128], f32, tag="sigT")
                nc.vector.tensor_copy(so[:], tp[:])
                nc.sync.dma_start(out_val[g * 128:(g + 1) * 128, :], so[:])

    nc.compile()
    return nc


_CACHE = {}


def kernel(z, edge_index, W1, b1, W2, b2):
    z = np.asarray(z, np.float32)
    edge_index = np.asarray(edge_index)
    W1 = np.asarray(W1, np.float32)
    W2 = np.asarray(W2, np.float32)
    b1 = np.asarray(b1, np.float32)
    b2 = np.asarray(b2, np.float32)
    if np.any(b1 != 0) or np.any(b2 != 0):
        return _host_reference(z, edge_index, W1, b1, W2, b2)

    import ml_dtypes
    from concourse import bass_utils
    bf16 = ml_dtypes.bfloat16

    plan = _plan(z, edge_index)
    key = (z.shape, edge_index.shape, plan['S'], tuple(plan['K'].tolist()))
    if key not in _CACHE:
        _CACHE.clear()
        _CACHE[key] = _build(plan, W1, W2)
    nc = _CACHE[key]

    w1t = np.ascontiguousarray(W1.T)
    iota_row = np.ascontiguousarray(
        np.tile(np.arange(128, dtype=np.float32), (128, 1))).astype(bf16)
    piota_col = np.arange(128, dtype=np.float32).reshape(128, 1)
    ones_row = np.ones((1, 128), np.float32).astype(bf16)
    in_maps = []
    for c in range(NC):
        in_maps.append({
            "z_cols": plan['z_cols'][c],
            "dinv_cols": plan['dinv_cols'][c],
            "dinv2_cols": plan['dinv2_cols'][c],
            "w1t": w1t, "w2": W2,
            "agg_idx": plan['agg_idx'][c],
            "adl_w": plan['adl_w'][c],
            "adl_flat": plan['adl_flat'][c],
            "iota_row": iota_row,
            "piota_col": piota_col,
            "ones_row": ones_row,
        })
    res = bass_utils.run_bass_kernel_spmd(nc, in_maps, core_ids=list(range(NC)))
    kernel._last = (nc, in_maps, plan)

    E = plan['E']
    flats = [res.results[c]["out_val"].reshape(-1) for c in range(NC)]
    out = np.empty(E, np.float32)
    oc, osl = plan['out_core'], plan['out_slot']
    for c in range(NC):
        m = oc == c
        out[m] = flats[c][osl[m]]
    return out


# revision 4
# speedup vs baseline: 1.1830x; 1.0182x over previous
"""GCN link-decoder kernel for 8 TRN2 NeuronCores — v2 (SWDGE-queue parallel).

Math (both GCNConv layers are linear, b1=b2=0): with P = D^-1/2 (A+I) D^-1/2,
    t0 = dinv*z; agg1 = scatter-sum t0[src]; t1 = dinv^2*(agg1 + t0)
    agg2 = scatter-sum t1[src]; u = dinv*(agg2 + t1); w = u @ G, G=(W1W2)(W1W2)^T
    val_e = u[src_e] . w[dst_e];  out = sigmoid(val)

Design notes (from the profiled baseline, 17.0 ms):
- The bottleneck was SWDGE descriptor generation on the GpSimd engine
  (~7.8 ns/gather-index, 13.1 ms of Pool time). v2 cuts per-edge gather
  streams from 4 to 3 (L1, L2, score-u) by computing the dst-side score
  factor w[dst_e] with a one-hot expansion matmul on TensorE, and spreads
  the remaining gathers across all 4 SWDGE queues, whose descriptor
  generation runs concurrently (measured 3-4x).
- Everything per-edge is processed in ONE slot layout shared by all three
  phases: cell = (src bucket, dst tile) at the dst-owner core, bucket-major
  slots, t-major processing (PSUM accumulates the 4 buckets per dst tile).
- Tables are bf16 (logits are in [-0.5, 0.7], so bf16 end-to-end error is
  ~1e-3 abs, far inside the 2e-2 gate); one-hots bf16 (2x DVE is_equal,
  FWL weight loads).
"""
import sys
import os
import bisect
sys.path.insert(0, '/opt/trn_rl_repo')
import numpy as np

NC = 8          # cores
P = 128         # partitions
FWH = 128       # table row width in bf16 elems (256B dma_gather granule)
BUCK = 32768    # int16 index bucket size (table rows per bucket)
BLK = 4096      # gather idxs per dma_gather instruction


def _wrap_idx16(arr: np.ndarray) -> np.ndarray:
    """Linear int16 slot-index array (len % 128 == 0) -> [128, len/16] SWDGE
    wrapped layout (slot k at partition k%16, col k//16; replicated to 128)."""
    n = arr.shape[0]
    t16 = arr.reshape(n // 16, 16).T
    return np.ascontiguousarray(np.tile(t16, (8, 1)))


def _host_reference(z, edge_index, W1, b1, W2, b2):
    """Numpy fallback (used only when b1/b2 are nonzero)."""
    N = z.shape[0]
    src, dst = edge_index[0], edge_index[1]
    deg = (np.bincount(dst, minlength=N) + 1.0).astype(np.float64)
    dinv = (1.0 / np.sqrt(deg)).astype(np.float32)

    def conv(x, W, b):
        h = x @ W
        out = np.zeros_like(h)
        np.add.at(out, dst, h[src] * (dinv[src] * dinv[dst])[:, None])
        out += h * (dinv * dinv)[:, None]
        return out + b

    h = conv(z, W1, b1)
    h = conv(h, W2, b2)
    val = np.einsum('ef,ef->e', h[src], h[dst]).astype(np.float64)
    return (1.0 / (1.0 + np.exp(-val))).astype(np.float32)


def _plan(z, edge_index):
    import ml_dtypes
    bf16 = ml_dtypes.bfloat16
    N = z.shape[0]
    E = edge_index.shape[1]
    assert N % NC == 0
    npc = N // NC
    npad = ((npc + P - 1) // P) * P
    tiles = npad // P
    nrows = NC * npad
    nbuck = (nrows + BUCK - 1) // BUCK

    src = edge_index[0].astype(np.int64)
    dst = edge_index[1].astype(np.int64)
    deg = np.bincount(dst, minlength=N).astype(np.float64) + 1.0
    dinv = (1.0 / np.sqrt(deg)).astype(np.float32)

    owner_s, local_s = src // npc, src % npc
    owner_d, local_d = dst // npc, dst % npc
    pid_s = (owner_s * npad + local_s).astype(np.int64)
    b_s = (pid_s // BUCK).astype(np.int64)
    t_d = local_d // P
    dstloc = local_d % P

    plan = {
        'N': N, 'E': E, 'npc': npc, 'npad': npad, 'tiles': tiles,
        'nrows': nrows, 'nbuck': nbuck, 'dinv': dinv,
    }

    # ---------------- slot layout (shared by L1/L2/score) -----------------
    cell = (b_s * tiles + t_d).astype(np.int64)   # bucket-major cell id
    ncell = nbuck * tiles
    counts = np.zeros((NC, ncell), np.int64)
    for c in range(NC):
        counts[c] = np.bincount(cell[owner_d == c], minlength=ncell)
    K = np.ceil(counts.max(axis=0) / P).astype(np.int64)
    cell_ofs = np.concatenate([[0], np.cumsum(K * P)])
    S = int(cell_ofs[-1])
    totc = S // P
    totc_pad = ((totc + P - 1) // P) * P
    plan['K'] = K
    plan['cell_ofs'] = cell_ofs
    plan['S'] = S
    plan['totc_pad'] = totc_pad

    agg_idx = np.zeros((NC, 128, S // 16), np.int16)
    adl_w = np.zeros((NC, 128, S // P), bf16)
    adl_flat = np.zeros((NC, 1, S), bf16)
    out_core = owner_d
    out_slot = np.zeros(E, np.int64)
    for c in range(NC):
        m = owner_d == c
        cl = cell[m]
        order = np.argsort(cl, kind='stable')
        cl_s = cl[order]
        grp_start = np.searchsorted(cl_s, np.arange(ncell))
        rank = np.arange(cl_s.shape[0]) - grp_start[cl_s]
        slot = cell_ofs[cl_s] + rank
        eidx = np.nonzero(m)[0][order]
        out_slot[eidx] = slot
        idx_lin = np.zeros(S, np.int16)
        dl_lin = np.full(S, -1.0, np.float32)
        idx_lin[slot] = (pid_s[eidx] % BUCK).astype(np.int16)
        dl_lin[slot] = dstloc[eidx].astype(np.float32)
        agg_idx[c] = _wrap_idx16(idx_lin)
        adl_w[c] = np.ascontiguousarray(dl_lin.reshape(-1, P).T).astype(bf16)
        adl_flat[c, 0] = dl_lin.astype(bf16)
    plan['agg_idx'] = agg_idx
    plan['adl_w'] = adl_w
    plan['adl_flat'] = adl_flat
    plan['out_core'] = out_core
    plan['out_slot'] = out_slot

    # gather blocks: contiguous slot ranges within one src bucket
    blocks = []  # (bucket, slot_start, n_idxs)
    for b in range(nbuck):
        s0 = int(cell_ofs[b * tiles])
        s1 = int(cell_ofs[(b + 1) * tiles])
        s = s0
        while s < s1:
            n = min(BLK, s1 - s)
            blocks.append((b, s, n))
            s += n
    plan['blocks'] = blocks

    # ---------------- per-core node data ----------------------------------
    z_cols = np.zeros((NC, 128, tiles * 16), np.float32)
    dinv_cols = np.zeros((NC, 128, tiles), np.float32)
    for c in range(NC):
        zc = np.zeros((npad, 16), np.float32)
        zc[:npc] = z[c * npc:(c + 1) * npc]
        dc = np.zeros(npad, np.float32)
        dc[:npc] = dinv[c * npc:(c + 1) * npc]
        z_cols[c] = zc.reshape(tiles, P, 16).transpose(1, 0, 2).reshape(P, tiles * 16)
        dinv_cols[c] = dc.reshape(tiles, P).T
    plan['z_cols'] = z_cols
    plan['dinv_cols'] = dinv_cols
    plan['dinv2_cols'] = dinv_cols * dinv_cols
    return plan


def _build(plan, W1np, W2np):
    from concourse import bass, bacc, tile, mybir
    from concourse.masks import make_identity

    npad, tiles, nrows, nbuck = plan['npad'], plan['tiles'], plan['nrows'], plan['nbuck']
    S, totc_pad = plan['S'], plan['totc_pad']
    K, cell_ofs = plan['K'], plan['cell_ofs']
    blocks = plan['blocks']
    f32 = mybir.dt.float32
    bf16 = mybir.dt.bfloat16
    i16 = mybir.dt.int16
    AF = mybir.ActivationFunctionType
    ALU = mybir.AluOpType

    nc = bacc.Bacc("TRN2", target_bir_lowering=False, debug=False,
                   num_devices=NC, num_swdge_queues=4)

    # ---- I/O ----
    in_z = nc.dram_tensor("z_cols", [128, tiles * 16], f32, kind="ExternalInput")
    in_dinv = nc.dram_tensor("dinv_cols", [128, tiles], f32, kind="ExternalInput")
    in_dinv2 = nc.dram_tensor("dinv2_cols", [128, tiles], f32, kind="ExternalInput")
    in_w1t = nc.dram_tensor("w1t", [256, 16], f32, kind="ExternalInput")
    in_w2 = nc.dram_tensor("w2", [256, 256], f32, kind="ExternalInput")
    in_idx = nc.dram_tensor("agg_idx", [128, S // 16], i16, kind="ExternalInput")
    in_adl = nc.dram_tensor("adl_w", [128, S // P], bf16, kind="ExternalInput")
    in_adlf = nc.dram_tensor("adl_flat", [1, S], bf16, kind="ExternalInput")
    in_iota = nc.dram_tensor("iota_row", [128, 128], bf16, kind="ExternalInput")
    in_piota = nc.dram_tensor("piota_col", [128, 1], f32, kind="ExternalInput")
    in_ones = nc.dram_tensor("ones_row", [1, 128], bf16, kind="ExternalInput")
    out_val = nc.dram_tensor("out_val", [totc_pad, 128], f32, kind="ExternalOutput")

    qctr = [0]

    def next_q():
        q = qctr[0] % 4
        qctr[0] += 1
        return q

    with tile.TileContext(nc) as tc:
        with tc.tile_pool(name="res", bufs=1) as res, \
             tc.tile_pool(name="gat", bufs=11) as gat, \
             tc.tile_pool(name="idx", bufs=12) as idxp, \
             tc.tile_pool(name="oh", bufs=6) as ohp, \
             tc.tile_pool(name="ohx", bufs=10) as ohxp, \
             tc.tile_pool(name="sm", bufs=2) as sm, \
             tc.tile_pool(name="pagg", bufs=4, space="PSUM") as pagg, \
             tc.tile_pool(name="ptp", bufs=4, space="PSUM") as ptp, \
             tc.tile_pool(name="dram", bufs=1, space="DRAM") as dram:

            # ================= phase 0: constants, t0, G ==================
            ident = res.tile([128, 128], f32)
            make_identity(nc, ident[:])

            # hoist num_idxs registers: one MOVE each, reused by every
            # dma_gather (a fresh to_reg per gather creates a WAR chain on
            # the shared register that serializes gather dispatch)
            nregs = {}

            def nreg(n):
                if n not in nregs:
                    nregs[n] = nc.gpsimd.to_reg(n)
                return nregs[n]
            iota = res.tile([128, 128], bf16)
            nc.sync.dma_start(iota[:], in_iota[:])
            piota = res.tile([128, 1], f32)
            nc.sync.dma_start(piota[:], in_piota[:])
            ones_r = res.tile([1, 128], bf16)
            nc.sync.dma_start(ones_r[:], in_ones[:])

            dinv_t = res.tile([128, tiles], f32)
            nc.sync.dma_start(dinv_t[:], in_dinv[:])
            dinv2_t = res.tile([128, tiles], f32)
            nc.sync.dma_start(dinv2_t[:], in_dinv2[:])

            adl_t = res.tile([128, S // P], bf16)
            nc.sync.dma_start(adl_t[:], in_adl[:])

            # t0 = dinv * z ; s1 = dinv^2 * t0 (pre-scaled self-loop for L1)
            t0_c = res.tile([128, tiles * 16], f32)
            nc.sync.dma_start(t0_c[:], in_z[:])
            nc.vector.tensor_tensor(
                out=t0_c[:].rearrange("p (t f) -> p t f", f=16),
                in0=t0_c[:].rearrange("p (t f) -> p t f", f=16),
                in1=dinv_t[:][:, :, None].to_broadcast([128, tiles, 16]),
                op=ALU.mult)
            s1_c = res.tile([128, tiles * 16], f32)
            nc.vector.tensor_tensor(
                out=s1_c[:].rearrange("p (t f) -> p t f", f=16),
                in0=t0_c[:].rearrange("p (t f) -> p t f", f=16),
                in1=dinv2_t[:][:, :, None].to_broadcast([128, tiles, 16]),
                op=ALU.mult)

            # G = (W1 @ W2) @ (W1 @ W2)^T  [16,16] f32
            w1t_s = res.tile([128, 2 * 16], f32)
            nc.sync.dma_start(w1t_s[:, 0:16], in_w1t[0:128, :])
            nc.sync.dma_start(w1t_s[:, 16:32], in_w1t[128:256, :])
            w2_s = res.tile([128, 2 * 256], f32)
            nc.sync.dma_start(w2_s[:, 0:256], in_w2[0:128, :])
            nc.sync.dma_start(w2_s[:, 256:512], in_w2[128:256, :])
            w12_ps = ptp.tile([16, 256], f32, tag="tp", space="PSUM")
            nc.tensor.matmul(w12_ps[:], lhsT=w1t_s[:, 0:16], rhs=w2_s[:, 0:256], start=True, stop=False)
            nc.tensor.matmul(w12_ps[:], lhsT=w1t_s[:, 16:32], rhs=w2_s[:, 256:512], start=False, stop=True)
            w12_s = res.tile([16, 256], f32)
            nc.vector.tensor_copy(w12_s[:], w12_ps[:])
            w12T_s = res.tile([128, 2 * 16], f32)
            for blkk in range(2):
                tp = ptp.tile([128, 16], f32, tag="tp", space="PSUM")
                nc.tensor.transpose(tp[:], in_=w12_s[:, blkk * 128:(blkk + 1) * 128], identity=ident[:16, :16])
                nc.vector.tensor_copy(w12T_s[:, blkk * 16:(blkk + 1) * 16], tp[:])
            g_ps = ptp.tile([16, 16], f32, tag="tp", space="PSUM")
            nc.tensor.matmul(g_ps[:], lhsT=w12T_s[:, 0:16], rhs=w12T_s[:, 0:16], start=True, stop=False)
            nc.tensor.matmul(g_ps[:], lhsT=w12T_s[:, 16:32], rhs=w12T_s[:, 16:32], start=False, stop=True)
            g_s = res.tile([16, 16], f32)
            nc.vector.tensor_copy(g_s[:], g_ps[:])

            # ---- table write helper (cols f32 -> bf16 rows 0:16 of bounce) ----
            def table_write(cols_f32, bounce):
                h = sm.tile([128, tiles * 16], bf16, tag="casth")
                nc.vector.tensor_copy(h[:], cols_f32[:])
                dst = bounce[:].rearrange("(t p) f -> p t f", p=128)[:, :, 0:16]
                nc.sync.dma_start(dst, h[:].rearrange("p (t f) -> p t f", f=16))

            rg = [list(range(NC))]
            t0b = dram.tile([npad, FWH], bf16)
            t0f = dram.tile([nrows, FWH], bf16)
            table_write(t0_c, t0b)
            nc.gpsimd.collective_compute(
                "AllGather", ALU.bypass,
                ins=[t0b.opt()], outs=[t0f.opt()], replica_groups=rg)
            gen_oh_l1 = None  # created below once agg helpers are defined

            # ---- lazy gather-block machinery (per phase) -----------------
            bstarts = [b[1] for b in blocks]

            def make_get_block(table_full):
                # t-major consumption touches all 4 buckets in lockstep;
                # cache <=2 live blocks per bucket so the pool (bufs=10)
                # never deadlocks on slot reuse (consumption is monotonic
                # per bucket, so evicted blocks are never re-requested).
                blk_tiles = {}
                per_bucket = {b: [] for b in range(nbuck)}

                def get_block(slot):
                    bi = bisect.bisect_right(bstarts, slot) - 1
                    if bi in blk_tiles:
                        return blk_tiles[bi]
                    b, s0, n = blocks[bi]
                    it = idxp.tile([128, BLK // 16], i16, tag="idx")
                    nc.sync.dma_start(it[:, :n // 16], in_idx[:, s0 // 16:(s0 + n) // 16])
                    gt = gat.tile([128, (BLK // 128) * FWH], bf16, tag="gtab")
                    lo = b * BUCK
                    hi = min(lo + BUCK, nrows)
                    nc.gpsimd.dma_gather(
                        out_ap=gt[:, :(n // 128) * FWH].rearrange("p (c f) -> p c f", f=FWH),
                        in_ap=table_full[lo:hi, :],
                        idxs_ap=it[:, :n // 16],
                        num_idxs=n, num_idxs_reg=nreg(n), elem_size=FWH,
                        single_packet=False, queue_num=next_q())
                    blk_tiles[bi] = (gt, s0)
                    per_bucket[b].append(bi)
                    if len(per_bucket[b]) > 2:
                        del blk_tiles[per_bucket[b].pop(0)]
                    return blk_tiles[bi]
                return get_block

            # ================= aggregation layer ==========================
            def make_gen_oh():
                # cell one-hots, cached FIFO-6 (creation order == consumption
                # order), so a few cells can be pre-generated to overlap the
                # preceding AllGather.
                cache = {}
                fifo = []

                def gen_oh(cidx):
                    if cidx in cache:
                        return cache[cidx]
                    kk = int(K[cidx])
                    c0 = int(cell_ofs[cidx])
                    oh = ohp.tile([128, kk * 128], bf16, tag="oh")
                    nc.vector.tensor_tensor(
                        out=oh[:].rearrange("p (k q) -> p k q", q=128),
                        in0=iota[:][:, None, :].to_broadcast([128, kk, 128]),
                        in1=adl_t[:, c0 // P:c0 // P + kk][:, :, None]
                            .to_broadcast([128, kk, 128]),
                        op=ALU.is_equal)
                    cache[cidx] = oh
                    fifo.append(cidx)
                    if len(fifo) > 5:
                        del cache[fifo.pop(0)]
                    return oh
                return gen_oh

            def prewarm_oh(gen_oh, n=6):
                done = 0
                for t in range(tiles):
                    for b in range(nbuck):
                        cidx = b * tiles + t
                        if int(K[cidx]) == 0:
                            continue
                        gen_oh(cidx)
                        done += 1
                        if done >= n:
                            return

            def agg_layer(table_full, selfloop_sc, scale_t, out_c, gen_oh):
                get_block = make_get_block(table_full)
                for t in range(tiles):
                    # collect (cell, j) list for this tile
                    mms = []
                    for b in range(nbuck):
                        cidx = b * tiles + t
                        kk = int(K[cidx])
                        if kk == 0:
                            continue
                        mms.append((cidx, kk))
                    pt = pagg.tile([16, 128], f32, tag="agg", space="PSUM")
                    total = sum(kk for _, kk in mms)
                    done = 0
                    for cidx, kk in mms:
                        c0 = int(cell_ofs[cidx])
                        oh = gen_oh(cidx)
                        for j in range(kk):
                            slot = c0 + j * 128
                            gt, s0 = get_block(slot)
                            ch = (slot - s0) // 128
                            nc.tensor.matmul(
                                pt[:],
                                lhsT=gt[:].rearrange("p (c f) -> p c f", f=FWH)[:, ch, 0:16],
                                rhs=oh[:, j * 128:(j + 1) * 128],
                                start=(done == 0), stop=(done == total - 1))
                            done += 1
                    # epilogue: out[:, t] = scale * aggP + selfloop_scaled
                    aggT = sm.tile([16, 128], f32, tag="aggT")
                    nc.scalar.activation(aggT[:], pt[:], AF.Copy)
                    tpp = ptp.tile([128, 16], f32, tag="tp", space="PSUM")
                    nc.tensor.transpose(tpp[:], in_=aggT[:], identity=ident[:16, :16])
                    nc.vector.scalar_tensor_tensor(
                        out=out_c[:, t * 16:(t + 1) * 16],
                        in0=tpp[:],
                        scalar=scale_t[:, t:t + 1],
                        in1=selfloop_sc[:, t * 16:(t + 1) * 16],
                        op0=ALU.mult, op1=ALU.add)

            # L1: t1 = dinv2*agg1 + s1
            gen_oh_l1 = make_gen_oh()
            prewarm_oh(gen_oh_l1)
            t1_c = res.tile([128, tiles * 16], f32)
            agg_layer(t0f, s1_c, dinv2_t, t1_c, gen_oh_l1)
            # L2 one-hot prewarm overlaps the T1 AllGather below
            gen_oh_l2 = make_gen_oh()
            t1b = dram.tile([npad, FWH], bf16)
            t1f = dram.tile([nrows, FWH], bf16)
            table_write(t1_c, t1b)
            prewarm_oh(gen_oh_l2)
            nc.gpsimd.collective_compute(
                "AllGather", ALU.bypass,
                ins=[t1b.opt()], outs=[t1f.opt()], replica_groups=rg)

            # L2: u = dinv*agg2 + dinv*t1
            s2_c = res.tile([128, tiles * 16], f32)
            nc.vector.tensor_tensor(
                out=s2_c[:].rearrange("p (t f) -> p t f", f=16),
                in0=t1_c[:].rearrange("p (t f) -> p t f", f=16),
                in1=dinv_t[:][:, :, None].to_broadcast([128, tiles, 16]),
                op=ALU.mult)
            u_c = res.tile([128, tiles * 16], f32)
            agg_layer(t1f, s2_c, dinv_t, u_c, gen_oh_l2)

            ub = dram.tile([npad, FWH], bf16)
            uf = dram.tile([nrows, FWH], bf16)
            table_write(u_c, ub)

            # w = u @ G per tile (bf16 cols for the uexp matmuls)
            w_c = res.tile([128, tiles * 16], bf16)
            for t in range(tiles):
                tpu = ptp.tile([16, 128], f32, tag="tp", space="PSUM")
                nc.tensor.transpose(tpu[:], in_=u_c[:, t * 16:(t + 1) * 16], identity=ident[:])
                uT = sm.tile([16, 128], f32, tag="uT")
                nc.scalar.activation(uT[:], tpu[:], AF.Copy)
                wp = ptp.tile([128, 16], f32, tag="tp", space="PSUM")
                nc.tensor.matmul(wp[:], lhsT=uT[:], rhs=g_s[:], start=True, stop=True)
                nc.scalar.activation(w_c[:, t * 16:(t + 1) * 16], wp[:], AF.Copy)

            # ================= scoring ====================================
            # OH_exp segments (lazy, 512 slots each):
            # replica = ones^T @ adl_flat (PE) -> is_equal(piota) off PSUM
            SEG = 512
            seg_tiles = {}
            seg_fifo = []

            def get_seg(slot):
                si = slot // SEG
                if si in seg_tiles:
                    return seg_tiles[si]
                s0 = si * SEG
                n = min(SEG, S - s0)
                adlf = idxp.tile([1, SEG], bf16, tag="adlf")
                nc.sync.dma_start(adlf[:, :n], in_adlf[:, s0:s0 + n])
                rep_ps = pagg.tile([128, SEG], f32, tag="agg", space="PSUM")
                nc.tensor.matmul(rep_ps[:, :n], lhsT=ones_r[:], rhs=adlf[:, :n],
                                 start=True, stop=True)
                ohx = ohxp.tile([128, SEG], bf16, tag="ohx")
                nc.vector.tensor_scalar(
                    out=ohx[:, :n],
                    in0=rep_ps[:, :n],
                    scalar1=piota[:, 0:1],
                    scalar2=None,
                    op0=ALU.is_equal)
                seg_tiles[si] = (ohx, s0)
                seg_fifo.append(si)
                if len(seg_fifo) > 9:
                    del seg_tiles[seg_fifo.pop(0)]
                return seg_tiles[si]

            # prewarm OH_exp segments in consumption order, then issue the
            # U AllGather so the segment chain overlaps the collective
            seen = []
            for t in range(tiles):
                for b in range(nbuck):
                    cidx = b * tiles + t
                    kk = int(K[cidx])
                    c0 = int(cell_ofs[cidx])
                    for j in range(kk):
                        si = (c0 + j * 128) // SEG
                        if si not in seen:
                            seen.append(si)
                if len(seen) >= 8:
                    break
            for si in seen[:9]:
                get_seg(si * SEG)
            nc.gpsimd.collective_compute(
                "AllGather", ALU.bypass,
                ins=[ub.opt()], outs=[uf.opt()], replica_groups=rg)

            get_score_block = make_get_block(uf)
            val = res.tile([128, totc_pad], f32)
            nc.vector.memset(val[:], 0.0)
            for t in range(tiles):
                for b in range(nbuck):
                    cidx = b * tiles + t
                    kk = int(K[cidx])
                    if kk == 0:
                        continue
                    c0 = int(cell_ofs[cidx])
                    uex = ptp.tile([128, kk * 16], f32, tag="tp", space="PSUM")
                    for j in range(kk):
                        slot = c0 + j * 128
                        ohx, s0 = get_seg(slot)
                        off = slot - s0
                        nc.tensor.matmul(
                            uex[:, j * 16:(j + 1) * 16],
                            lhsT=ohx[:, off:off + 128],
                            rhs=w_c[:, t * 16:(t + 1) * 16],
                            start=True, stop=True)
                    # dot with gathered u[src] rows, split on block straddle
                    j = 0
                    while j < kk:
                        slot = c0 + j * 128
                        gt, s0 = get_score_block(slot)
                        ch = (slot - s0) // 128
                        # how many chunks stay inside this block?
                        room = (s0 + BLK - slot) // 128
                        m = min(kk - j, room)
                        prod = sm.tile([128, kk * 16], f32, tag="prod")
                        nc.vector.tensor_tensor(
                            out=prod[:, j * 16:(j + m) * 16].rearrange("p (c f) -> p c f", f=16),
                            in0=uex[:, j * 16:(j + m) * 16].rearrange("p (c f) -> p c f", f=16),
                            in1=gt[:].rearrange("p (c f) -> p c f", f=FWH)[:, ch:ch + m, 0:16],
                            op=ALU.mult)
                        nc.vector.reduce_sum(
                            out=val[:, c0 // P + j:c0 // P + j + m],
                            in_=prod[:, j * 16:(j + m) * 16].rearrange("p (c f) -> p c f", f=16),
                            axis=mybir.AxisListType.X)
                        j += m

            # sigmoid + transpose + out
            for g in range(totc_pad // 128):
                sg = sm.tile([128, 128], f32, tag="sig")
                nc.scalar.activation(sg[:], val[:, g * 128:(g + 1) * 128], AF.Sigmoid)
                tp = ptp.tile([128, 128], f32, tag="tp", space="PSUM")
                nc.tensor.transpose(tp[:], in_=sg[:], identity=ident[:])
                so = sm.tile([128, 128], f32, tag="sigT")
                nc.vector.tensor_copy(so[:], tp[:])
                nc.sync.dma_start(out_val[g * 128:(g + 1) * 128, :], so[:])

    nc.compile()
    return nc


_CACHE = {}


def kernel(z, edge_index, W1, b1, W2, b2):
    z = np.asarray(z, np.float32)
    edge_index = np.asarray(edge_index)
    W1 = np.asarray(W1, np.float32)
    W2 = np.asarray(W2, np.float32)
    b1 = np.asarray(b1, np.float32)
    b2 = np.asarray(b2, np.float32)
    if np.any(b1 != 0) or np.any(b2 != 0):
        return _host_reference(z, edge_index, W1, b1, W2, b2)

    import ml_dtypes
    from concourse import bass_utils
    bf16 = ml_dtypes.bfloat16

    plan = _plan(z, edge_index)
    key = (z.shape, edge_index.shape, plan['S'], tuple(plan['K'].tolist()))
    if key not in _CACHE:
        _CACHE.clear()
        _CACHE[key] = _build(plan, W1, W2)
    nc = _CACHE[key]

    w1t = np.ascontiguousarray(W1.T)
    iota_row = np.ascontiguousarray(
        np.tile(np.arange(128, dtype=np.float32), (128, 1))).astype(bf16)
    piota_col = np.arange(128, dtype=np.float32).reshape(128, 1)
    ones_row = np.ones((1, 128), np.float32).astype(bf16)
    in_maps = []
    for c in range(NC):
        in_maps.append({
            "z_cols": plan['z_cols'][c],
            "dinv_cols": plan['dinv_cols'][c],
            "dinv2_cols": plan['dinv2_cols'][c],
            "w1t": w1t, "w2": W2,
            "agg_idx": plan['agg_idx'][c],
            "adl_w": plan['adl_w'][c],
            "adl_flat": plan['adl_flat'][c],
            "iota_row": iota_row,
            "piota_col": piota_col,
            "ones_row": ones_row,
        })
    res = bass_utils.run_bass_kernel_spmd(nc, in_maps, core_ids=list(range(NC)))
    kernel._last = (nc, in_maps, plan)

    E = plan['E']
    flats = [res.results[c]["out_val"].reshape(-1) for c in range(NC)]
    out = np.empty(E, np.float32)
    oc, osl = plan['out_core'], plan['out_slot']
    for c in range(NC):
        m = oc == c
        out[m] = flats[c][osl[m]]
    return out


# revision 6
# speedup vs baseline: 1.2408x; 1.0488x over previous
"""GCN link-decoder kernel for 8 TRN2 NeuronCores — v2 (SWDGE-queue parallel).

Math (both GCNConv layers are linear, b1=b2=0): with P = D^-1/2 (A+I) D^-1/2,
    t0 = dinv*z; agg1 = scatter-sum t0[src]; t1 = dinv^2*(agg1 + t0)
    agg2 = scatter-sum t1[src]; u = dinv*(agg2 + t1); w = u @ G, G=(W1W2)(W1W2)^T
    val_e = u[src_e] . w[dst_e];  out = sigmoid(val)

Design notes (from the profiled baseline, 17.0 ms):
- The bottleneck was SWDGE descriptor generation on the GpSimd engine
  (~7.8 ns/gather-index, 13.1 ms of Pool time). v2 cuts per-edge gather
  streams from 4 to 3 (L1, L2, score-u) by computing the dst-side score
  factor w[dst_e] with a one-hot expansion matmul on TensorE, and spreads
  the remaining gathers across all 4 SWDGE queues, whose descriptor
  generation runs concurrently (measured 3-4x).
- Everything per-edge is processed in ONE slot layout shared by all three
  phases: cell = (src bucket, dst tile) at the dst-owner core, bucket-major
  slots, t-major processing (PSUM accumulates the 4 buckets per dst tile).
- Tables are bf16 (logits are in [-0.5, 0.7], so bf16 end-to-end error is
  ~1e-3 abs, far inside the 2e-2 gate); one-hots bf16 (2x DVE is_equal,
  FWL weight loads).
"""
import sys
import os
import bisect
sys.path.insert(0, '/opt/trn_rl_repo')
import numpy as np

NC = 8          # cores
P = 128         # partitions
FWH = 128       # table row width in bf16 elems (256B dma_gather granule)
BUCK = 32768    # int16 index bucket size (table rows per bucket)
BLK = 2048      # gather idxs per dma_gather instruction


def _wrap_idx16(arr: np.ndarray) -> np.ndarray:
    """Linear int16 slot-index array (len % 128 == 0) -> [128, len/16] SWDGE
    wrapped layout (slot k at partition k%16, col k//16; replicated to 128)."""
    n = arr.shape[0]
    t16 = arr.reshape(n // 16, 16).T
    return np.ascontiguousarray(np.tile(t16, (8, 1)))


def _host_reference(z, edge_index, W1, b1, W2, b2):
    """Numpy fallback (used only when b1/b2 are nonzero)."""
    N = z.shape[0]
    src, dst = edge_index[0], edge_index[1]
    deg = (np.bincount(dst, minlength=N) + 1.0).astype(np.float64)
    dinv = (1.0 / np.sqrt(deg)).astype(np.float32)

    def conv(x, W, b):
        h = x @ W
        out = np.zeros_like(h)
        np.add.at(out, dst, h[src] * (dinv[src] * dinv[dst])[:, None])
        out += h * (dinv * dinv)[:, None]
        return out + b

    h = conv(z, W1, b1)
    h = conv(h, W2, b2)
    val = np.einsum('ef,ef->e', h[src], h[dst]).astype(np.float64)
    return (1.0 / (1.0 + np.exp(-val))).astype(np.float32)


def _plan(z, edge_index):
    import ml_dtypes
    bf16 = ml_dtypes.bfloat16
    N = z.shape[0]
    E = edge_index.shape[1]
    assert N % NC == 0
    npc = N // NC
    npad = ((npc + P - 1) // P) * P
    tiles = npad // P
    nrows = NC * npad
    nbuck = (nrows + BUCK - 1) // BUCK

    src = edge_index[0].astype(np.int64)
    dst = edge_index[1].astype(np.int64)
    deg = np.bincount(dst, minlength=N).astype(np.float64) + 1.0
    dinv = (1.0 / np.sqrt(deg)).astype(np.float32)

    owner_s, local_s = src // npc, src % npc
    owner_d, local_d = dst // npc, dst % npc
    pid_s = (owner_s * npad + local_s).astype(np.int64)
    b_s = (pid_s // BUCK).astype(np.int64)
    t_d = local_d // P
    dstloc = local_d % P

    plan = {
        'N': N, 'E': E, 'npc': npc, 'npad': npad, 'tiles': tiles,
        'nrows': nrows, 'nbuck': nbuck, 'dinv': dinv,
    }

    # ---------------- slot layout (shared by L1/L2/score) -----------------
    cell = (b_s * tiles + t_d).astype(np.int64)   # bucket-major cell id
    ncell = nbuck * tiles
    counts = np.zeros((NC, ncell), np.int64)
    for c in range(NC):
        counts[c] = np.bincount(cell[owner_d == c], minlength=ncell)
    K = np.ceil(counts.max(axis=0) / P).astype(np.int64)
    cell_ofs = np.concatenate([[0], np.cumsum(K * P)])
    S = int(cell_ofs[-1])
    totc = S // P
    totc_pad = ((totc + P - 1) // P) * P
    plan['K'] = K
    plan['cell_ofs'] = cell_ofs
    plan['S'] = S
    plan['totc_pad'] = totc_pad

    agg_idx = np.zeros((NC, 128, S // 16), np.int16)
    adl_w = np.zeros((NC, 128, S // P), bf16)
    adl_flat = np.zeros((NC, 1, S), bf16)
    out_core = owner_d
    out_slot = np.zeros(E, np.int64)
    for c in range(NC):
        m = owner_d == c
        cl = cell[m]
        order = np.argsort(cl, kind='stable')
        cl_s = cl[order]
        grp_start = np.searchsorted(cl_s, np.arange(ncell))
        rank = np.arange(cl_s.shape[0]) - grp_start[cl_s]
        slot = cell_ofs[cl_s] + rank
        eidx = np.nonzero(m)[0][order]
        out_slot[eidx] = slot
        idx_lin = np.zeros(S, np.int16)
        dl_lin = np.full(S, -1.0, np.float32)
        idx_lin[slot] = (pid_s[eidx] % BUCK).astype(np.int16)
        dl_lin[slot] = dstloc[eidx].astype(np.float32)
        agg_idx[c] = _wrap_idx16(idx_lin)
        adl_w[c] = np.ascontiguousarray(dl_lin.reshape(-1, P).T).astype(bf16)
        adl_flat[c, 0] = dl_lin.astype(bf16)
    plan['agg_idx'] = agg_idx
    plan['adl_w'] = adl_w
    plan['adl_flat'] = adl_flat
    plan['out_core'] = out_core
    plan['out_slot'] = out_slot

    # gather blocks: contiguous slot ranges within one src bucket
    blocks = []  # (bucket, slot_start, n_idxs)
    for b in range(nbuck):
        s0 = int(cell_ofs[b * tiles])
        s1 = int(cell_ofs[(b + 1) * tiles])
        s = s0
        while s < s1:
            n = min(BLK, s1 - s)
            blocks.append((b, s, n))
            s += n
    plan['blocks'] = blocks

    # ---------------- per-core node data ----------------------------------
    z_cols = np.zeros((NC, 128, tiles * 16), np.float32)
    dinv_cols = np.zeros((NC, 128, tiles), np.float32)
    for c in range(NC):
        zc = np.zeros((npad, 16), np.float32)
        zc[:npc] = z[c * npc:(c + 1) * npc]
        dc = np.zeros(npad, np.float32)
        dc[:npc] = dinv[c * npc:(c + 1) * npc]
        z_cols[c] = zc.reshape(tiles, P, 16).transpose(1, 0, 2).reshape(P, tiles * 16)
        dinv_cols[c] = dc.reshape(tiles, P).T
    plan['z_cols'] = z_cols
    plan['dinv_cols'] = dinv_cols
    plan['dinv2_cols'] = dinv_cols * dinv_cols
    return plan


def _build(plan, W1np, W2np):
    from concourse import bass, bacc, tile, mybir
    from concourse.masks import make_identity

    npad, tiles, nrows, nbuck = plan['npad'], plan['tiles'], plan['nrows'], plan['nbuck']
    S, totc_pad = plan['S'], plan['totc_pad']
    K, cell_ofs = plan['K'], plan['cell_ofs']
    blocks = plan['blocks']
    f32 = mybir.dt.float32
    bf16 = mybir.dt.bfloat16
    i16 = mybir.dt.int16
    AF = mybir.ActivationFunctionType
    ALU = mybir.AluOpType

    nc = bacc.Bacc("TRN2", target_bir_lowering=False, debug=False,
                   num_devices=NC, num_swdge_queues=4)

    # ---- I/O ----
    in_z = nc.dram_tensor("z_cols", [128, tiles * 16], f32, kind="ExternalInput")
    in_dinv = nc.dram_tensor("dinv_cols", [128, tiles], f32, kind="ExternalInput")
    in_dinv2 = nc.dram_tensor("dinv2_cols", [128, tiles], f32, kind="ExternalInput")
    in_w1t = nc.dram_tensor("w1t", [256, 16], f32, kind="ExternalInput")
    in_w2 = nc.dram_tensor("w2", [256, 256], f32, kind="ExternalInput")
    in_idx = nc.dram_tensor("agg_idx", [128, S // 16], i16, kind="ExternalInput")
    in_adl = nc.dram_tensor("adl_w", [128, S // P], bf16, kind="ExternalInput")
    in_adlf = nc.dram_tensor("adl_flat", [1, S], bf16, kind="ExternalInput")
    in_iota = nc.dram_tensor("iota_row", [128, 128], bf16, kind="ExternalInput")
    in_piota = nc.dram_tensor("piota_col", [128, 1], f32, kind="ExternalInput")
    in_ones = nc.dram_tensor("ones_row", [1, 128], bf16, kind="ExternalInput")
    out_val = nc.dram_tensor("out_val", [totc_pad, 128], f32, kind="ExternalOutput")

    qctr = [0]

    def next_q():
        q = qctr[0] % 4
        qctr[0] += 1
        return q

    with tile.TileContext(nc) as tc:
        with tc.tile_pool(name="res", bufs=1) as res, \
             tc.tile_pool(name="gat", bufs=20) as gat, \
             tc.tile_pool(name="idx", bufs=12) as idxp, \
             tc.tile_pool(name="oh", bufs=6) as ohp, \
             tc.tile_pool(name="ohx", bufs=10) as ohxp, \
             tc.tile_pool(name="sm", bufs=2) as sm, \
             tc.tile_pool(name="pagg", bufs=4, space="PSUM") as pagg, \
             tc.tile_pool(name="ptp", bufs=4, space="PSUM") as ptp, \
             tc.tile_pool(name="dram", bufs=1, space="DRAM") as dram:

            # ================= phase 0: constants, t0, G ==================
            ident = res.tile([128, 128], f32)
            make_identity(nc, ident[:])

            # hoist num_idxs registers: one MOVE each, reused by every
            # dma_gather (a fresh to_reg per gather creates a WAR chain on
            # the shared register that serializes gather dispatch)
            nregs = {}

            def nreg(n):
                if n not in nregs:
                    nregs[n] = nc.gpsimd.to_reg(n)
                return nregs[n]
            iota = res.tile([128, 128], bf16)
            nc.sync.dma_start(iota[:], in_iota[:])
            piota = res.tile([128, 1], f32)
            nc.sync.dma_start(piota[:], in_piota[:])
            ones_r = res.tile([1, 128], bf16)
            nc.sync.dma_start(ones_r[:], in_ones[:])

            dinv_t = res.tile([128, tiles], f32)
            nc.sync.dma_start(dinv_t[:], in_dinv[:])
            dinv2_t = res.tile([128, tiles], f32)
            nc.sync.dma_start(dinv2_t[:], in_dinv2[:])

            adl_t = res.tile([128, S // P], bf16)
            nc.sync.dma_start(adl_t[:], in_adl[:])

            # t0 = dinv * z ; s1 = dinv^2 * t0 (pre-scaled self-loop for L1)
            t0_c = res.tile([128, tiles * 16], f32)
            nc.sync.dma_start(t0_c[:], in_z[:])
            nc.vector.tensor_tensor(
                out=t0_c[:].rearrange("p (t f) -> p t f", f=16),
                in0=t0_c[:].rearrange("p (t f) -> p t f", f=16),
                in1=dinv_t[:][:, :, None].to_broadcast([128, tiles, 16]),
                op=ALU.mult)
            s1_c = res.tile([128, tiles * 16], f32)
            nc.vector.tensor_tensor(
                out=s1_c[:].rearrange("p (t f) -> p t f", f=16),
                in0=t0_c[:].rearrange("p (t f) -> p t f", f=16),
                in1=dinv2_t[:][:, :, None].to_broadcast([128, tiles, 16]),
                op=ALU.mult)

            # G = (W1 @ W2) @ (W1 @ W2)^T  [16,16] f32
            w1t_s = res.tile([128, 2 * 16], f32)
            nc.sync.dma_start(w1t_s[:, 0:16], in_w1t[0:128, :])
            nc.sync.dma_start(w1t_s[:, 16:32], in_w1t[128:256, :])
            w2_s = res.tile([128, 2 * 256], f32)
            nc.sync.dma_start(w2_s[:, 0:256], in_w2[0:128, :])
            nc.sync.dma_start(w2_s[:, 256:512], in_w2[128:256, :])
            w12_ps = ptp.tile([16, 256], f32, tag="tp", space="PSUM")
            nc.tensor.matmul(w12_ps[:], lhsT=w1t_s[:, 0:16], rhs=w2_s[:, 0:256], start=True, stop=False)
            nc.tensor.matmul(w12_ps[:], lhsT=w1t_s[:, 16:32], rhs=w2_s[:, 256:512], start=False, stop=True)
            w12_s = res.tile([16, 256], f32)
            nc.vector.tensor_copy(w12_s[:], w12_ps[:])
            w12T_s = res.tile([128, 2 * 16], f32)
            for blkk in range(2):
                tp = ptp.tile([128, 16], f32, tag="tp", space="PSUM")
                nc.tensor.transpose(tp[:], in_=w12_s[:, blkk * 128:(blkk + 1) * 128], identity=ident[:16, :16])
                nc.vector.tensor_copy(w12T_s[:, blkk * 16:(blkk + 1) * 16], tp[:])
            g_ps = ptp.tile([16, 16], f32, tag="tp", space="PSUM")
            nc.tensor.matmul(g_ps[:], lhsT=w12T_s[:, 0:16], rhs=w12T_s[:, 0:16], start=True, stop=False)
            nc.tensor.matmul(g_ps[:], lhsT=w12T_s[:, 16:32], rhs=w12T_s[:, 16:32], start=False, stop=True)
            g_s = res.tile([16, 16], f32)
            nc.vector.tensor_copy(g_s[:], g_ps[:])

            # ---- table write helper (cols f32 -> bf16 rows 0:16 of bounce) ----
            def table_write(cols_f32, bounce):
                h = sm.tile([128, tiles * 16], bf16, tag="casth")
                nc.vector.tensor_copy(h[:], cols_f32[:])
                dst = bounce[:].rearrange("(t p) f -> p t f", p=128)[:, :, 0:16]
                nc.sync.dma_start(dst, h[:].rearrange("p (t f) -> p t f", f=16))

            rg = [list(range(NC))]
            t0b = dram.tile([npad, FWH], bf16)
            t0f = dram.tile([nrows, FWH], bf16)
            table_write(t0_c, t0b)
            nc.gpsimd.collective_compute(
                "AllGather", ALU.bypass,
                ins=[t0b.opt()], outs=[t0f.opt()], replica_groups=rg)
            gen_oh_l1 = None  # created below once agg helpers are defined

            # ---- lazy gather-block machinery (per phase) -----------------
            bstarts = [b[1] for b in blocks]

            def make_get_block(table_full):
                # t-major consumption touches all 4 buckets in lockstep;
                # cache <=2 live blocks per bucket so the pool (bufs=10)
                # never deadlocks on slot reuse (consumption is monotonic
                # per bucket, so evicted blocks are never re-requested).
                blk_tiles = {}
                per_bucket = {b: [] for b in range(nbuck)}

                def get_block(slot):
                    bi = bisect.bisect_right(bstarts, slot) - 1
                    if bi in blk_tiles:
                        return blk_tiles[bi]
                    b, s0, n = blocks[bi]
                    it = idxp.tile([128, BLK // 16], i16, tag="idx")
                    nc.sync.dma_start(it[:, :n // 16], in_idx[:, s0 // 16:(s0 + n) // 16])
                    gt = gat.tile([128, (BLK // 128) * FWH], bf16, tag="gtab")
                    lo = b * BUCK
                    hi = min(lo + BUCK, nrows)
                    nc.gpsimd.dma_gather(
                        out_ap=gt[:, :(n // 128) * FWH].rearrange("p (c f) -> p c f", f=FWH),
                        in_ap=table_full[lo:hi, :],
                        idxs_ap=it[:, :n // 16],
                        num_idxs=n, num_idxs_reg=nreg(n), elem_size=FWH,
                        single_packet=False, queue_num=next_q())
                    blk_tiles[bi] = (gt, s0)
                    per_bucket[b].append(bi)
                    if len(per_bucket[b]) > 2:
                        del blk_tiles[per_bucket[b].pop(0)]
                    return blk_tiles[bi]
                return get_block

            # ================= aggregation layer ==========================
            def make_gen_oh():
                # cell one-hots, cached FIFO-6 (creation order == consumption
                # order), so a few cells can be pre-generated to overlap the
                # preceding AllGather.
                cache = {}
                fifo = []

                def gen_oh(cidx):
                    if cidx in cache:
                        return cache[cidx]
                    kk = int(K[cidx])
                    c0 = int(cell_ofs[cidx])
                    oh = ohp.tile([128, kk * 128], bf16, tag="oh")
                    nc.vector.tensor_tensor(
                        out=oh[:].rearrange("p (k q) -> p k q", q=128),
                        in0=iota[:][:, None, :].to_broadcast([128, kk, 128]),
                        in1=adl_t[:, c0 // P:c0 // P + kk][:, :, None]
                            .to_broadcast([128, kk, 128]),
                        op=ALU.is_equal)
                    cache[cidx] = oh
                    fifo.append(cidx)
                    if len(fifo) > 5:
                        del cache[fifo.pop(0)]
                    return oh
                return gen_oh

            def prewarm_oh(gen_oh, n=6):
                done = 0
                for t in range(tiles):
                    for b in range(nbuck):
                        cidx = b * tiles + t
                        if int(K[cidx]) == 0:
                            continue
                        gen_oh(cidx)
                        done += 1
                        if done >= n:
                            return

            def agg_layer(table_full, selfloop_sc, scale_t, out_c, gen_oh):
                get_block = make_get_block(table_full)
                for t in range(tiles):
                    # collect (cell, j) list for this tile
                    mms = []
                    for b in range(nbuck):
                        cidx = b * tiles + t
                        kk = int(K[cidx])
                        if kk == 0:
                            continue
                        mms.append((cidx, kk))
                    pt = pagg.tile([16, 128], f32, tag="agg", space="PSUM")
                    total = sum(kk for _, kk in mms)
                    done = 0
                    for cidx, kk in mms:
                        c0 = int(cell_ofs[cidx])
                        oh = gen_oh(cidx)
                        for j in range(kk):
                            slot = c0 + j * 128
                            gt, s0 = get_block(slot)
                            ch = (slot - s0) // 128
                            nc.tensor.matmul(
                                pt[:],
                                lhsT=gt[:].rearrange("p (c f) -> p c f", f=FWH)[:, ch, 0:16],
                                rhs=oh[:, j * 128:(j + 1) * 128],
                                start=(done == 0), stop=(done == total - 1))
                            done += 1
                    # epilogue: out[:, t] = scale * aggP + selfloop_scaled
                    aggT = sm.tile([16, 128], f32, tag="aggT")
                    nc.scalar.activation(aggT[:], pt[:], AF.Copy)
                    tpp = ptp.tile([128, 16], f32, tag="tp", space="PSUM")
                    nc.tensor.transpose(tpp[:], in_=aggT[:], identity=ident[:16, :16])
                    nc.vector.scalar_tensor_tensor(
                        out=out_c[:, t * 16:(t + 1) * 16],
                        in0=tpp[:],
                        scalar=scale_t[:, t:t + 1],
                        in1=selfloop_sc[:, t * 16:(t + 1) * 16],
                        op0=ALU.mult, op1=ALU.add)

            # L1: t1 = dinv2*agg1 + s1
            gen_oh_l1 = make_gen_oh()
            prewarm_oh(gen_oh_l1)
            t1_c = res.tile([128, tiles * 16], f32)
            agg_layer(t0f, s1_c, dinv2_t, t1_c, gen_oh_l1)
            # L2 one-hot prewarm overlaps the T1 AllGather below
            gen_oh_l2 = make_gen_oh()
            t1b = dram.tile([npad, FWH], bf16)
            t1f = dram.tile([nrows, FWH], bf16)
            table_write(t1_c, t1b)
            prewarm_oh(gen_oh_l2)
            nc.gpsimd.collective_compute(
                "AllGather", ALU.bypass,
                ins=[t1b.opt()], outs=[t1f.opt()], replica_groups=rg)

            # L2: u = dinv*agg2 + dinv*t1
            s2_c = res.tile([128, tiles * 16], f32)
            nc.vector.tensor_tensor(
                out=s2_c[:].rearrange("p (t f) -> p t f", f=16),
                in0=t1_c[:].rearrange("p (t f) -> p t f", f=16),
                in1=dinv_t[:][:, :, None].to_broadcast([128, tiles, 16]),
                op=ALU.mult)
            u_c = res.tile([128, tiles * 16], f32)
            agg_layer(t1f, s2_c, dinv_t, u_c, gen_oh_l2)

            ub = dram.tile([npad, FWH], bf16)
            uf = dram.tile([nrows, FWH], bf16)
            table_write(u_c, ub)

            # w = u @ G per tile (bf16 cols for the uexp matmuls)
            w_c = res.tile([128, tiles * 16], bf16)
            for t in range(tiles):
                tpu = ptp.tile([16, 128], f32, tag="tp", space="PSUM")
                nc.tensor.transpose(tpu[:], in_=u_c[:, t * 16:(t + 1) * 16], identity=ident[:])
                uT = sm.tile([16, 128], f32, tag="uT")
                nc.scalar.activation(uT[:], tpu[:], AF.Copy)
                wp = ptp.tile([128, 16], f32, tag="tp", space="PSUM")
                nc.tensor.matmul(wp[:], lhsT=uT[:], rhs=g_s[:], start=True, stop=True)
                nc.scalar.activation(w_c[:, t * 16:(t + 1) * 16], wp[:], AF.Copy)

            # ================= scoring ====================================
            # OH_exp segments (lazy, 512 slots each):
            # replica = ones^T @ adl_flat (PE) -> is_equal(piota) off PSUM
            SEG = 512
            seg_tiles = {}
            seg_fifo = []

            def get_seg(slot):
                si = slot // SEG
                if si in seg_tiles:
                    return seg_tiles[si]
                s0 = si * SEG
                n = min(SEG, S - s0)
                adlf = idxp.tile([1, SEG], bf16, tag="adlf")
                nc.sync.dma_start(adlf[:, :n], in_adlf[:, s0:s0 + n])
                rep_ps = pagg.tile([128, SEG], f32, tag="agg", space="PSUM")
                nc.tensor.matmul(rep_ps[:, :n], lhsT=ones_r[:], rhs=adlf[:, :n],
                                 start=True, stop=True)
                ohx = ohxp.tile([128, SEG], bf16, tag="ohx")
                nc.vector.tensor_scalar(
                    out=ohx[:, :n],
                    in0=rep_ps[:, :n],
                    scalar1=piota[:, 0:1],
                    scalar2=None,
                    op0=ALU.is_equal)
                seg_tiles[si] = (ohx, s0)
                seg_fifo.append(si)
                if len(seg_fifo) > 9:
                    del seg_tiles[seg_fifo.pop(0)]
                return seg_tiles[si]

            # prewarm OH_exp segments in consumption order, then issue the
            # U AllGather so the segment chain overlaps the collective
            seen = []
            for t in range(tiles):
                for b in range(nbuck):
                    cidx = b * tiles + t
                    kk = int(K[cidx])
                    c0 = int(cell_ofs[cidx])
                    for j in range(kk):
                        si = (c0 + j * 128) // SEG
                        if si not in seen:
                            seen.append(si)
                if len(seen) >= 8:
                    break
            for si in seen[:9]:
                get_seg(si * SEG)
            nc.gpsimd.collective_compute(
                "AllGather", ALU.bypass,
                ins=[ub.opt()], outs=[uf.opt()], replica_groups=rg)

            get_score_block = make_get_block(uf)
            val = res.tile([128, totc_pad], f32)
            nc.vector.memset(val[:], 0.0)
            for t in range(tiles):
                for b in range(nbuck):
                    cidx = b * tiles + t
                    kk = int(K[cidx])
                    if kk == 0:
                        continue
                    c0 = int(cell_ofs[cidx])
                    uex = ptp.tile([128, kk * 16], f32, tag="tp", space="PSUM")
                    for j in range(kk):
                        slot = c0 + j * 128
                        ohx, s0 = get_seg(slot)
                        off = slot - s0
                        nc.tensor.matmul(
                            uex[:, j * 16:(j + 1) * 16],
                            lhsT=ohx[:, off:off + 128],
                            rhs=w_c[:, t * 16:(t + 1) * 16],
                            start=True, stop=True)
                    # dot with gathered u[src] rows, split on block straddle
                    j = 0
                    while j < kk:
                        slot = c0 + j * 128
                        gt, s0 = get_score_block(slot)
                        ch = (slot - s0) // 128
                        # how many chunks stay inside this block?
                        room = (s0 + BLK - slot) // 128
                        m = min(kk - j, room)
                        prod = sm.tile([128, kk * 16], f32, tag="prod")
                        nc.vector.tensor_tensor(
                            out=prod[:, j * 16:(j + m) * 16].rearrange("p (c f) -> p c f", f=16),
                            in0=uex[:, j * 16:(j + m) * 16].rearrange("p (c f) -> p c f", f=16),
                            in1=gt[:].rearrange("p (c f) -> p c f", f=FWH)[:, ch:ch + m, 0:16],
                            op=ALU.mult)
                        nc.vector.reduce_sum(
                            out=val[:, c0 // P + j:c0 // P + j + m],
                            in_=prod[:, j * 16:(j + m) * 16].rearrange("p (c f) -> p c f", f=16),
                            axis=mybir.AxisListType.X)
                        j += m

            # sigmoid + transpose + out
            for g in range(totc_pad // 128):
                sg = sm.tile([128, 128], f32, tag="sig")
                nc.scalar.activation(sg[:], val[:, g * 128:(g + 1) * 128], AF.Sigmoid)
                tp = ptp.tile([128, 128], f32, tag="tp", space="PSUM")
                nc.tensor.transpose(tp[:], in_=sg[:], identity=ident[:])
                so = sm.tile([128, 128], f32, tag="sigT")
                nc.vector.tensor_copy(so[:], tp[:])
                nc.sync.dma_start(out_val[g * 128:(g + 1) * 128, :], so[:])

    nc.compile()
    return nc


_CACHE = {}


def kernel(z, edge_index, W1, b1, W2, b2):
    z = np.asarray(z, np.float32)
    edge_index = np.asarray(edge_index)
    W1 = np.asarray(W1, np.float32)
    W2 = np.asarray(W2, np.float32)
    b1 = np.asarray(b1, np.float32)
    b2 = np.asarray(b2, np.float32)
    if np.any(b1 != 0) or np.any(b2 != 0):
        return _host_reference(z, edge_index, W1, b1, W2, b2)

    import ml_dtypes
    from concourse import bass_utils
    bf16 = ml_dtypes.bfloat16

    plan = _plan(z, edge_index)
    key = (z.shape, edge_index.shape, plan['S'], tuple(plan['K'].tolist()))
    if key not in _CACHE:
        _CACHE.clear()
        _CACHE[key] = _build(plan, W1, W2)
    nc = _CACHE[key]

    w1t = np.ascontiguousarray(W1.T)
    iota_row = np.ascontiguousarray(
        np.tile(np.arange(128, dtype=np.float32), (128, 1))).astype(bf16)
    piota_col = np.arange(128, dtype=np.float32).reshape(128, 1)
    ones_row = np.ones((1, 128), np.float32).astype(bf16)
    in_maps = []
    for c in range(NC):
        in_maps.append({
            "z_cols": plan['z_cols'][c],
            "dinv_cols": plan['dinv_cols'][c],
            "dinv2_cols": plan['dinv2_cols'][c],
            "w1t": w1t, "w2": W2,
            "agg_idx": plan['agg_idx'][c],
            "adl_w": plan['adl_w'][c],
            "adl_flat": plan['adl_flat'][c],
            "iota_row": iota_row,
            "piota_col": piota_col,
            "ones_row": ones_row,
        })
    res = bass_utils.run_bass_kernel_spmd(nc, in_maps, core_ids=list(range(NC)))
    kernel._last = (nc, in_maps, plan)

    E = plan['E']
    flats = [res.results[c]["out_val"].reshape(-1) for c in range(NC)]
    out = np.empty(E, np.float32)
    oc, osl = plan['out_core'], plan['out_slot']
    for c in range(NC):
        m = oc == c
        out[m] = flats[c][osl[m]]
    return out
